# revision 24
# baseline (speedup 1.0000x reference)
"""BrahmaAttention (GQA prefill with KV cache) on 8 Trainium2 NeuronCores.

Problem: B=4, S=1024, C=1024 (cache), H=16 q-heads, G=4 kv-heads, D=128.
    q = hs @ wq.T ; k = hs @ wk.T ; v = hs @ wv.T
    rope(q, k) (interleaved pairs, positions C..C+S)
    k_full/v_full = concat(cache, new)           # K = 2048 keys
    out = softmax(q k^T / sqrt(D)) @ v_full @ wo.T
(attention_mask is all-zeros by construction - full attention, no masking.)

Sharding: 4-way data parallel over batch x 2-way tensor parallel over heads.
core (b, hg) handles batch b, q-heads hg*8..hg*8+8, kv-heads hg*2..hg*2+2 and
computes a partial output projection over its 1024 hidden columns; the host
sums the two partials per batch (the TP all-reduce done on host at gather).

Host-side prep folded into the shards:
  - 1/sqrt(D) folded into wq.
  - RoPE even/odd interleave permuted to [evens|odds] via wq/wk row
    permutation and cache_k last-dim permutation, so on-chip RoPE is
    half-tile elementwise ops (partitions 0-63 = even, 64-127 = odd lanes).
  - All projection weights pre-transposed/tiled so every DMA is contiguous,
    and shipped in bf16 (PE streams bf16 at the same 1 col/cycle as f32r,
    so bf16 costs nothing on PE and halves DMA + SBUF).

On-chip structure per core (all matmul operands bf16, PSUM f32):
  phase 1: q/k/v projections (PE) + rope (DVE + ACT-copy swap)
  phase 2: scoresT = kT.T @ qT -> PSUM -> exp (ACT) -> probs bf16
           softmax denominator: DVE pair-sum tree over the 16 key chunks,
           then ONE GpSimd partition_all_reduce => per-s denominator already
           broadcast to all partitions (zero PE cycles, zero bcast matmul)
           AV accumulated on PE; normalize fused into the PSUM->SBUF copy
           wo projection; s-half 0's wo tiles interleaved between s-half 1's
           heads to fill PE while ACT paces the exp chain
  cross-iteration software pipeline: hsT/wk/wq tiles double-buffered in
  always-open pools; the NEXT iteration's input DMAs are emitted on the SP
  queue before this iteration's output DMAs, so they stream during phase 2
  instead of serializing at the iteration boundary.  kv-cache loads ride the
  ACT queue (free after its last exp).
"""

import numpy as np
import ml_dtypes

B, S, C, H, G, D = 4, 1024, 1024, 16, 4, 128
HID = H * D
P = 128
NH, NG = 8, 2          # per-core q heads / kv heads
KC = (C + S) // P      # 16 key chunks
KT = 16                # hid contraction tiles
SH = 512               # s-half (PSUM bank free size)
N_CORES = 8

_PERM = np.concatenate([np.arange(0, D, 2), np.arange(1, D, 2)])

_BUILT = {}


def _mm(nc, out, lhsT, rhs, **kw):
    nc.tensor.matmul(out, lhsT, rhs, **kw)


def _rope(nc, pool, f32, psum_in, out_ap, cs_cc, cs_pm, mult):
    """out = psum_in*[cos;cos] + swap_halves(psum_in*[sin;-sin]).

    psum_in is the raw projected [128, S] tile with evens on partitions 0-63
    and odds on 64-127; out gets the roped value in the same layout.
    """
    import concourse.mybir as mybir

    a = pool.tile([P, S], f32, tag="ropeA", name="ropeA")
    b = pool.tile([P, S], f32, tag="ropeB", name="ropeB")
    s = pool.tile([P, S], f32, tag="ropeS", name="ropeS")
    nc.vector.tensor_tensor(a[:], psum_in[:], cs_cc[:], mult)
    nc.vector.tensor_tensor(b[:], psum_in[:], cs_pm[:], mult)
    # swap halves on the scalar engine (idle during phase 1)
    nc.scalar.copy(s[0:64, :], b[64:128, :])
    nc.scalar.copy(s[64:128, :], b[0:64, :])
    nc.vector.tensor_tensor(out_ap, a[:], s[:], mybir.AluOpType.add)


def build_bass(unroll=1):
    """Build + compile the per-core Bass program (identical on all cores)."""
    if unroll in _BUILT:
        return _BUILT[unroll]

    import concourse.mybir as mybir
    import concourse.tile as tile
    import concourse.bass_isa as bass_isa
    from concourse import bacc

    f32 = mybir.dt.float32
    f32r = mybir.dt.float32r
    bf16 = mybir.dt.bfloat16
    mult = mybir.AluOpType.mult
    add = mybir.AluOpType.add
    Exp = mybir.ActivationFunctionType.Exp

    nc = bacc.Bacc("TRN2", target_bir_lowering=False, debug=False)

    hsT_d = nc.dram_tensor("hsT", [KT, P, S], bf16, kind="ExternalInput")
    wq_d = nc.dram_tensor("wqT", [NH, KT, P, P], bf16, kind="ExternalInput")
    wk_d = nc.dram_tensor("wkT", [NG, KT, P, P], bf16, kind="ExternalInput")
    wv_d = nc.dram_tensor("wvT", [KT, P, NG * P], bf16, kind="ExternalInput")
    wo_d = nc.dram_tensor("woT", [NH, P, HID], bf16, kind="ExternalInput")
    ck_d = nc.dram_tensor("ckT", [NG, P, C], bf16, kind="ExternalInput")
    cv_d = nc.dram_tensor("cvP", [P, C // P, NG, P], bf16, kind="ExternalInput")
    cc_d = nc.dram_tensor("cs_cc", [P, S], f32, kind="ExternalInput")
    pm_d = nc.dram_tensor("cs_pm", [P, S], f32, kind="ExternalInput")
    y_d = nc.dram_tensor("y", [S, HID], bf16, kind="ExternalOutput")

    with tile.TileContext(nc) as tc:
        with (
            tc.tile_pool(name="const", bufs=1) as const,
            tc.tile_pool(name="hs", bufs=2) as hs_pool,
            tc.tile_pool(name="wq_pool", bufs=2) as wq_pool,
            tc.tile_pool(name="wk_pool", bufs=2) as wk_pool,
            tc.tile_pool(name="a1", bufs=2) as a1_pool,
            tc.tile_pool(name="persist", bufs=1) as persist,
        ):
            ones_f = const.tile([P, P], f32, name="ones_f")
            nc.any.memset(ones_f[:], 1.0)
            ones128 = const.tile([P, P], f32r, name="ones128")
            nc.vector.tensor_copy(ones128[:], ones_f[:])
            cs_cc = const.tile([P, S], f32, name="cs_cc")
            cs_pm = const.tile([P, S], f32, name="cs_pm")

            qT = persist.tile([P, NH, S], bf16, name="qT")
            kT = persist.tile([P, NG, C + S], bf16, name="kT")
            vF = persist.tile([P, KC, NG, P], bf16, name="vF")
            attn0 = persist.tile([P, NH, SH], bf16, name="attn0")

            # weights identical across unrolled iterations: wk and wo are
            # loaded once up front and never reloaded.
            wks = []
            for g in range(NG):
                wk = wk_pool.tile([P, KT, P], bf16, tag="wk", name="wk_sb")
                nc.sync.dma_start(wk[:],
                                  wk_d[g].rearrange("k p m -> p k m"))
                wks.append(wk)
            wons = []
            for n in range(HID // SH):
                won = persist.tile([P, NH, SH], bf16, tag=f"won{n}",
                                   name="won")
                nc.sync.dma_start(
                    won[:],
                    wo_d[:, :, n * SH:(n + 1) * SH]
                    .rearrange("h p n -> p h n"),
                )
                wons.append(won)

            env = dict(
                nc=nc, tc=tc, f32=f32, f32r=f32r, bf16=bf16, mult=mult,
                add=add, Exp=Exp, bass_isa=bass_isa,
                hsT_d=hsT_d, wq_d=wq_d, wk_d=wk_d, wv_d=wv_d, wo_d=wo_d,
                ck_d=ck_d, cv_d=cv_d, cc_d=cc_d, pm_d=pm_d, y_d=y_d,
                hs_pool=hs_pool, wq_pool=wq_pool, wk_pool=wk_pool,
                qT=qT, kT=kT, vF=vF, attn0=attn0,
                wks=wks, wons=wons,
                ones128=ones128, cs_cc=cs_cc, cs_pm=cs_pm,
                attn1_pool=a1_pool,
            )

            # iteration 0's phase-1 loads, emitted cold
            pre = _emit_prefetch(env)
            prev_wo1 = None
            for it in range(unroll):
                pre, prev_wo1 = _emit_iteration(
                    env, it, pre, prefetch_next=(it + 1 < unroll),
                    prev_wo1=prev_wo1)

    nc.compile()
    _BUILT[unroll] = nc
    return nc


def _emit_prefetch(env):
    """Emit the early input loads for one iteration on the SP queue:
    hsT (all 16 k-tiles) and wq heads 0-1.  Returns the tiles for the
    consuming iteration."""
    nc = env["nc"]
    bf16 = env["bf16"]
    hsT = env["hs_pool"].tile([P, KT, S], bf16, tag="hsT", name="hsT_sb")
    for i in range(8):
        nc.sync.dma_start(
            hsT[:, 2 * i:2 * i + 2, :],
            env["hsT_d"][2 * i:2 * i + 2].rearrange("k p s -> p k s"),
        )
    wqs = {}
    for h in range(2):
        wq = env["wq_pool"].tile([P, KT, P], bf16, tag="wq", name="wq_sb")
        nc.sync.dma_start(wq[:], env["wq_d"][h].rearrange("k p m -> p k m"))
        wqs[h] = wq
    return {"hsT": hsT, "wqs": wqs}


def _emit_iteration(env, it, pre, prefetch_next, prev_wo1=None):
    import concourse.tile as tile  # noqa: F401

    nc = env["nc"]
    tc = env["tc"]
    f32, bf16 = env["f32"], env["bf16"]
    mult, add, Exp = env["mult"], env["add"], env["Exp"]
    bass_isa = env["bass_isa"]
    qT, kT, vF = env["qT"], env["kT"], env["vF"]
    attn0 = env["attn0"]
    cs_cc, cs_pm = env["cs_cc"], env["cs_pm"]
    hsT, wqs = pre["hsT"], pre["wqs"]
    wks, wons = env["wks"], env["wons"]
    y_d = env["y_d"]

    # ---------------- phase 1: projections + rope ----------------
    with (
        tc.tile_pool(name="rope", bufs=1) as rope_pool,
        tc.tile_pool(name="ps1", bufs=1, space="PSUM") as ps1,
    ):
        if it == 0:
            # PE warm-up during the initial DMA window; rope tables; caches.
            pw = ps1.tile([P, P], f32, tag="warm", bufs=1, name="pwarm")
            for i in range(150):
                _mm(nc, pw[:], env["ones128"][:], env["ones128"][:],
                    start=(i == 0), stop=(i == 149), skip_group_check=True)
            wsink = rope_pool.tile([1, 1], f32, tag="wsink", name="wsink")
            nc.vector.tensor_copy(wsink[:], pw[0:1, 0:1])
            nc.sync.dma_start(cs_cc[:], env["cc_d"][:])
            nc.sync.dma_start(cs_pm[:], env["pm_d"][:])
            for g in range(NG):
                nc.sync.dma_start(kT[:, g, 0:C], env["ck_d"][g])
            nc.sync.dma_start(vF[:, 0:C // P, :, :], env["cv_d"][:])

        wv = rope_pool.tile([P, KT, NG * P], bf16, tag="wv", name="wv_sb")
        nc.sync.dma_start(wv[:], env["wv_d"].rearrange("k p n -> p k n"))

        # k projection + rope (new keys go to kT[:, g, C:])
        for g in range(NG):
            pk = ps1.tile([P, S], f32, tag="pqk", bufs=2, name="pk")
            for k in range(KT):
                for n in range(2):
                    _mm(nc, pk[:, n * SH:(n + 1) * SH], wks[g][:, k, :],
                        hsT[:, k, n * SH:(n + 1) * SH],
                        start=(k == 0), stop=(k == KT - 1))
            _rope(nc, rope_pool, f32, pk, kT[:, g, C:C + S], cs_cc, cs_pm,
                  mult)

        # q projection + rope
        for h in range(NH):
            if h in wqs:
                wq = wqs[h]
            else:
                wq = env["wq_pool"].tile([P, KT, P], bf16, tag="wq",
                                         name="wq_sb")
                nc.sync.dma_start(wq[:],
                                  env["wq_d"][h].rearrange("k p m -> p k m"))
            pq = ps1.tile([P, S], f32, tag="pqk", bufs=2, name="pq")
            for k in range(KT):
                for n in range(2):
                    _mm(nc, pq[:, n * SH:(n + 1) * SH], wq[:, k, :],
                        hsT[:, k, n * SH:(n + 1) * SH],
                        start=(k == 0), stop=(k == KT - 1))
            _rope(nc, rope_pool, f32, pq, qT[:, h, :], cs_cc, cs_pm, mult)

        # v projection (natural layout: tokens on partitions)
        for mv in range(S // P):
            pv = ps1.tile([P, NG * P], f32, tag="pv", bufs=2, name="pv")
            for k in range(KT):
                _mm(nc, pv[:], hsT[:, k, mv * P:(mv + 1) * P], wv[:, k, :],
                    start=(k == 0), stop=(k == KT - 1))
            nc.vector.tensor_copy(vF[:, C // P + mv, :, :], pv[:])

    # next iteration's early loads: on the SP queue BEFORE this iteration's
    # y-output triggers, so they stream during this phase 2.
    nxt = _emit_prefetch(env) if prefetch_next else None

    # ---------------- phase 2: attention + output projection ----------------
    with (
        tc.tile_pool(name="probs", bufs=1) as probs_pool,
        tc.tile_pool(name="small", bufs=2) as small_pool,
        tc.tile_pool(name="ps2", bufs=1, space="PSUM") as ps2,
    ):
        attn1 = env["attn1_pool"].tile([P, NH, SH], bf16, tag="attn1",
                                       name="attn1_sb")

        class WoEmitter:
            """Emits one s-half's output projection as a stream of single
            matmuls so they can be interleaved into the attention pipeline
            as PE filler (the exp chain on ACT otherwise paces PE)."""

            def __init__(self, attn_t, sh):
                self.attn_t = attn_t
                self.sh = sh
                self.jobs = [(n, mt) for n in range(HID // SH)
                             for mt in range(4)]
                self.ji = 0
                self.hi = 0
                self.py = None

            def exhausted(self):
                return self.ji >= len(self.jobs)

            def emit_one(self):
                if self.exhausted():
                    return False
                n, mt = self.jobs[self.ji]
                if self.hi == 0:
                    self.py = ps2.tile([P, SH], f32, tag="py", bufs=3,
                                       name="py")
                h = self.hi
                _mm(nc, self.py[:],
                    self.attn_t[:, h, mt * P:(mt + 1) * P],
                    wons[n][:, h, :], start=(h == 0), stop=(h == NH - 1),
                    skip_group_check=True)
                self.hi += 1
                if self.hi == NH:
                    ysb = small_pool.tile([P, SH], bf16, tag="ysb",
                                          name="ysb")
                    nc.vector.tensor_copy(ysb[:], self.py[:])
                    m = self.sh * 4 + mt
                    nc.sync.dma_start(
                        y_d[m * P:(m + 1) * P, n * SH:(n + 1) * SH], ysb[:],
                    )
                    self.hi = 0
                    self.ji += 1
                return True

            def emit_some(self, k):
                for _ in range(k):
                    if not self.emit_one():
                        return

            def emit_all(self):
                while self.emit_one():
                    pass

        def attention_head(sh, h, attn_t, filler):
            ssl = slice(sh * SH, (sh + 1) * SH)
            g = h // (NH // NG)
            NCG = KC // 2  # chunk groups of 2
            probs = [None] * NCG
            d1 = [None] * NCG
            d2 = [None] * (NCG // 2)
            d3 = [None] * (NCG // 4)
            pav_box = [None]
            den_bc = small_pool.tile([P, SH], f32, tag="denbc", name="den_bc")

            def emit_scores(cg):
                ps = ps2.tile([P, 2, SH], f32, tag="score", bufs=2,
                              name="pscore")
                for j in range(2):
                    c = cg * 2 + j
                    _mm(nc, ps[:, j, :], kT[:, g, c * P:(c + 1) * P],
                        qT[:, h, ssl], start=True, stop=True)
                pt = probs_pool.tile([P, 2, SH], bf16, tag="probs",
                                     bufs=4, name="probs_t")
                nc.scalar.activation(pt[:], ps[:], Exp)
                probs[cg] = pt
                # denominator: DVE pair-sum tree (bf16), one GpSimd
                # partition_all_reduce at the end -> per-s denominator
                # broadcast to every partition; zero PE cycles.
                t1 = probs_pool.tile([P, SH], bf16, tag="d1", bufs=2,
                                     name="d1_t")
                nc.vector.tensor_tensor(t1[:], pt[:, 0, :], pt[:, 1, :], add)
                d1[cg] = t1
                if cg % 2 == 1:
                    t2 = probs_pool.tile([P, SH], bf16, tag="d2", bufs=2,
                                         name="d2_t")
                    nc.vector.tensor_tensor(t2[:], d1[cg - 1][:], t1[:], add)
                    d2[cg // 2] = t2
                if cg % 4 == 3:
                    t3 = probs_pool.tile([P, SH], bf16, tag="d3", bufs=2,
                                         name="d3_t")
                    nc.vector.tensor_tensor(t3[:], d2[cg // 2 - 1][:],
                                            d2[cg // 2][:], add)
                    d3[cg // 4] = t3
                if cg == NCG - 1:
                    t4 = probs_pool.tile([P, SH], bf16, tag="d4", bufs=1,
                                         name="d4_t")
                    nc.vector.tensor_tensor(t4[:], d3[0][:], d3[1][:], add)
                    nc.gpsimd.partition_all_reduce(
                        den_bc[:], t4[:], channels=P,
                        reduce_op=bass_isa.ReduceOp.add)

            def emit_av(cg):
                # lazy: score tiles grab the low PSUM banks, which phase 1's
                # first tiles reuse -- scores free at the last exp, not at
                # the last head's normalize, so the next iteration's
                # projections start ~3.5us earlier.
                if pav_box[0] is None:
                    pav_box[0] = ps2.tile([P, SH], f32, tag="av", bufs=1,
                                          name="pav")
                pav = pav_box[0]
                for j in range(2):
                    c = cg * 2 + j
                    first, last = (c == 0), (c == KC - 1)
                    _mm(nc, pav[:], vF[:, c, g, :], probs[cg][:, j, :],
                        start=first, stop=last, skip_group_check=True)

            # software pipeline: scores 2 groups ahead of AV; up to 2 wo
            # filler matmuls per step keep PE fed while ACT works on exp
            for cg in range(NCG + 2):
                if filler is not None:
                    filler.emit_some(2)
                if cg < NCG:
                    emit_scores(cg)
                if cg >= 2:
                    emit_av(cg - 2)

            # reciprocal of the broadcast denominator, fused into the AV
            # PSUM->SBUF copyback
            rbc = small_pool.tile([P, SH], f32, tag="rbc", name="rbc")
            nc.vector.reciprocal_approx_fast(out=rbc[:], in_=den_bc[:])
            nc.vector.tensor_tensor(attn_t[:, h, :], pav_box[0][:], rbc[:],
                                    mult)

        # s-half 0 heads, filled with the PREVIOUS iteration's s-half-1
        # output projection; s-half 1 heads, filled with this iteration's
        # s-half-0 output projection.
        fill0 = WoEmitter(prev_wo1, 1) if prev_wo1 is not None else None
        for h in range(NH):
            attention_head(0, h, attn0, fill0)
        if fill0 is not None:
            fill0.emit_all()
        fill1 = WoEmitter(attn0, 0)
        for h in range(NH):
            attention_head(1, h, attn1, fill1)
        fill1.emit_all()
        if not prefetch_next:
            # last iteration: its s-half-1 wo has no later home
            WoEmitter(attn1, 1).emit_all()

        # next iteration's kv-cache loads ride the ACT queue: ACT reaches
        # them right after its last exp, and kT/vF are free then too.
        if prefetch_next:
            for g in range(NG):
                nc.scalar.dma_start(kT[:, g, 0:C], env["ck_d"][g])
            nc.scalar.dma_start(vF[:, 0:C // P, :, :], env["cv_d"][:])

    return nxt, attn1


def prep_inputs(hidden_states, freqs_cos, freqs_sin, cache_k, cache_v,
                wq, wk, wv, wo):
    """Shard + pre-transpose the full inputs into 8 per-core input maps."""
    f = np.float32
    b16 = ml_dtypes.bfloat16
    scale = np.float32(1.0 / np.sqrt(D))
    wq_p = (wq.astype(f).reshape(H, D, HID)[:, _PERM, :] * scale)
    wk_p = wk.astype(f).reshape(G, D, HID)[:, _PERM, :]
    wv_r = wv.astype(f).reshape(G, D, HID)

    cc = freqs_cos.astype(f).T          # [64, S]
    ss = freqs_sin.astype(f).T
    cs_cc = np.ascontiguousarray(np.concatenate([cc, cc], axis=0))
    cs_pm = np.ascontiguousarray(np.concatenate([ss, -ss], axis=0))

    in_maps = []
    for b in range(B):
        hsT = np.ascontiguousarray(
            hidden_states[b].astype(f).T.reshape(KT, P, S)).astype(b16)
        for hg in range(2):
            hs_q = slice(hg * NH, (hg + 1) * NH)
            hs_kv = slice(hg * NG, (hg + 1) * NG)
            wqT = wq_p[hs_q].reshape(NH * D, HID).T          # [HID, 1024]
            wqT_t = np.ascontiguousarray(
                wqT.reshape(KT, P, NH, P).transpose(2, 0, 1, 3)).astype(b16)
            wkT = wk_p[hs_kv].reshape(NG * D, HID).T         # [HID, 256]
            wkT_t = np.ascontiguousarray(
                wkT.reshape(KT, P, NG, P).transpose(2, 0, 1, 3)).astype(b16)
            wvT = wv_r[hs_kv].reshape(NG * D, HID).T         # [HID, 256]
            wvT_t = np.ascontiguousarray(
                wvT.reshape(KT, P, NG * P)).astype(b16)
            woT = np.ascontiguousarray(
                wo.astype(f)[:, hg * NH * D:(hg + 1) * NH * D].T
                .reshape(NH, P, HID)).astype(b16)
            ckT = np.ascontiguousarray(
                cache_k[b].astype(f)[:, hs_kv][:, :, _PERM]
                .transpose(1, 2, 0)).astype(b16)
            cvP = np.ascontiguousarray(
                cache_v[b].astype(f)[:, hs_kv]
                .reshape(C // P, P, NG, P).transpose(1, 0, 2, 3)).astype(b16)
            in_maps.append({
                "hsT": hsT, "wqT": wqT_t, "wkT": wkT_t, "wvT": wvT_t,
                "woT": woT, "ckT": ckT, "cvP": cvP,
                "cs_cc": cs_cc, "cs_pm": cs_pm,
            })
    return in_maps


def gather_output(results):
    """Sum the 2 TP partials per batch -> full [B, S, HID] output."""
    out = np.empty((B, S, HID), np.float32)
    for b in range(B):
        out[b] = results[2 * b]["y"] + results[2 * b + 1]["y"]
    return out


def kernel(hidden_states, freqs_cos, freqs_sin, attention_mask,
           cache_k, cache_v, wq, wk, wv, wo):
    # attention_mask is all-zeros by construction (see spec) - unused.
    from concourse.bass_utils import run_bass_kernel_spmd

    nc = build_bass(unroll=1)
    in_maps = prep_inputs(
        np.asarray(hidden_states), np.asarray(freqs_cos), np.asarray(freqs_sin),
        np.asarray(cache_k), np.asarray(cache_v),
        np.asarray(wq), np.asarray(wk), np.asarray(wv), np.asarray(wo))
    res = run_bass_kernel_spmd(nc, in_maps, core_ids=list(range(N_CORES)))
    return gather_output(res.results)


# revision 27
# speedup vs baseline: 1.1556x; 1.1556x over previous
"""BrahmaAttention (GQA prefill with KV cache) on 8 Trainium2 NeuronCores.

Problem: B=4, S=1024, C=1024 (cache), H=16 q-heads, G=4 kv-heads, D=128.
    q = hs @ wq.T ; k = hs @ wk.T ; v = hs @ wv.T
    rope(q, k) (interleaved pairs, positions C..C+S)
    k_full/v_full = concat(cache, new)           # K = 2048 keys
    out = softmax(q k^T / sqrt(D)) @ v_full @ wo.T
(attention_mask is all-zeros by construction - full attention, no masking.)

Sharding: 4-way data parallel over batch x 2-way tensor parallel over heads.
core (b, hg) handles batch b, q-heads hg*8..hg*8+8, kv-heads hg*2..hg*2+2 and
computes a partial output projection over its 1024 hidden columns; the host
sums the two partials per batch (the TP all-reduce done on host at gather).

Host-side prep folded into the shards:
  - 1/sqrt(D) folded into wq.
  - RoPE even/odd interleave permuted to [evens|odds] via wq/wk row
    permutation and cache_k last-dim permutation, so on-chip RoPE is
    half-tile elementwise ops (partitions 0-63 = even, 64-127 = odd lanes).
  - All projection weights pre-transposed/tiled so every DMA is contiguous,
    and shipped in bf16 (PE streams bf16 at the same 1 col/cycle as f32r,
    so bf16 costs nothing on PE and halves DMA + SBUF).

On-chip structure per core (all matmul operands bf16, PSUM f32):
  phase 1: q/k/v projections (PE) + rope (DVE + ACT-copy swap)
  phase 2: scoresT = kT.T @ qT -> PSUM -> exp (ACT) -> probs bf16
           softmax denominator: DVE pair-sum tree over the 16 key chunks,
           then ONE GpSimd partition_all_reduce => per-s denominator already
           broadcast to all partitions (zero PE cycles, zero bcast matmul)
           AV accumulated on PE; normalize fused into the PSUM->SBUF copy
           wo projection; s-half 0's wo tiles interleaved between s-half 1's
           heads to fill PE while ACT paces the exp chain
  cross-iteration software pipeline: hsT/wk/wq tiles double-buffered in
  always-open pools; the NEXT iteration's input DMAs are emitted on the SP
  queue before this iteration's output DMAs, so they stream during phase 2
  instead of serializing at the iteration boundary.  kv-cache loads ride the
  ACT queue (free after its last exp).
"""

import numpy as np
import ml_dtypes

B, S, C, H, G, D = 4, 1024, 1024, 16, 4, 128
HID = H * D
P = 128
NH, NG = 8, 2          # per-core q heads / kv heads
KC = (C + S) // P      # 16 key chunks
KT = 16                # hid contraction tiles
SH = 512               # s-half (PSUM bank free size)
N_CORES = 8

_PERM = np.concatenate([np.arange(0, D, 2), np.arange(1, D, 2)])

_BUILT = {}


def _mm(nc, out, lhsT, rhs, **kw):
    nc.tensor.matmul(out, lhsT, rhs, **kw)


def _rope(nc, pool, f32, psum_in, out_ap, cs_cc, cs_pm, mult):
    """out = psum_in*[cos;cos] + swap_halves(psum_in*[sin;-sin]).

    psum_in is the raw projected [128, S] tile with evens on partitions 0-63
    and odds on 64-127; out gets the roped value in the same layout.
    """
    import concourse.mybir as mybir

    a = pool.tile([P, S], f32, tag="ropeA", name="ropeA")
    b = pool.tile([P, S], f32, tag="ropeB", name="ropeB")
    s = pool.tile([P, S], f32, tag="ropeS", name="ropeS")
    nc.vector.tensor_tensor(a[:], psum_in[:], cs_cc[:], mult)
    nc.vector.tensor_tensor(b[:], psum_in[:], cs_pm[:], mult)
    # swap halves on the scalar engine (idle during phase 1)
    nc.scalar.copy(s[0:64, :], b[64:128, :])
    nc.scalar.copy(s[64:128, :], b[0:64, :])
    nc.vector.tensor_tensor(out_ap, a[:], s[:], mybir.AluOpType.add)


def build_bass(unroll=1):
    """Build + compile the per-core Bass program (identical on all cores)."""
    if unroll in _BUILT:
        return _BUILT[unroll]

    import concourse.mybir as mybir
    import concourse.tile as tile
    import concourse.bass_isa as bass_isa
    from concourse import bacc

    f32 = mybir.dt.float32
    f32r = mybir.dt.float32r
    bf16 = mybir.dt.bfloat16
    mult = mybir.AluOpType.mult
    add = mybir.AluOpType.add
    Exp = mybir.ActivationFunctionType.Exp

    nc = bacc.Bacc("TRN2", target_bir_lowering=False, debug=False)

    hsT_d = nc.dram_tensor("hsT", [KT, P, S], bf16, kind="ExternalInput")
    wq_d = nc.dram_tensor("wqT", [NH, KT, P, P], bf16, kind="ExternalInput")
    wk_d = nc.dram_tensor("wkT", [NG, KT, P, P], bf16, kind="ExternalInput")
    wv_d = nc.dram_tensor("wvT", [KT, P, NG * P], bf16, kind="ExternalInput")
    wo_d = nc.dram_tensor("woT", [NH, P, HID], bf16, kind="ExternalInput")
    ck_d = nc.dram_tensor("ckT", [NG, P, C], bf16, kind="ExternalInput")
    cv_d = nc.dram_tensor("cvP", [P, C // P, NG, P], bf16, kind="ExternalInput")
    cc_d = nc.dram_tensor("cs_cc", [P, S], f32, kind="ExternalInput")
    pm_d = nc.dram_tensor("cs_pm", [P, S], f32, kind="ExternalInput")
    y_d = nc.dram_tensor("y", [S, HID], bf16, kind="ExternalOutput")

    with tile.TileContext(nc) as tc:
        with (
            tc.tile_pool(name="const", bufs=1) as const,
            tc.tile_pool(name="hs", bufs=2) as hs_pool,
            tc.tile_pool(name="wq_pool", bufs=2) as wq_pool,
            tc.tile_pool(name="wk_pool", bufs=2) as wk_pool,
            tc.tile_pool(name="a1", bufs=2) as a1_pool,
            tc.tile_pool(name="persist", bufs=1) as persist,
            tc.tile_pool(name="psum", bufs=1, space="PSUM") as psum_pool,
        ):
            ones_f = const.tile([P, P], f32, name="ones_f")
            nc.any.memset(ones_f[:], 1.0)
            ones128 = const.tile([P, P], f32r, name="ones128")
            nc.vector.tensor_copy(ones128[:], ones_f[:])
            cs_cc = const.tile([P, S], f32, name="cs_cc")
            cs_pm = const.tile([P, S], f32, name="cs_pm")

            qT = persist.tile([P, NH, S], bf16, name="qT")
            kT = persist.tile([P, NG, C + S], bf16, name="kT")
            vF = persist.tile([P, KC, NG, P], bf16, name="vF")
            attn0 = persist.tile([P, NH, SH], bf16, name="attn0")

            # weights identical across unrolled iterations: wk and wo are
            # loaded once up front and never reloaded.
            wks = []
            for g in range(NG):
                wk = wk_pool.tile([P, KT, P], bf16, tag="wk", name="wk_sb")
                nc.sync.dma_start(wk[:],
                                  wk_d[g].rearrange("k p m -> p k m"))
                wks.append(wk)
            wons = [persist.tile([P, NH, SH], bf16, tag=f"won{n}",
                                 name="won") for n in range(HID // SH)]

            env = dict(
                nc=nc, tc=tc, f32=f32, f32r=f32r, bf16=bf16, mult=mult,
                add=add, Exp=Exp, bass_isa=bass_isa,
                hsT_d=hsT_d, wq_d=wq_d, wk_d=wk_d, wv_d=wv_d, wo_d=wo_d,
                ck_d=ck_d, cv_d=cv_d, cc_d=cc_d, pm_d=pm_d, y_d=y_d,
                hs_pool=hs_pool, wq_pool=wq_pool, wk_pool=wk_pool,
                qT=qT, kT=kT, vF=vF, attn0=attn0,
                wks=wks, wons=wons,
                ones128=ones128, cs_cc=cs_cc, cs_pm=cs_pm,
                attn1_pool=a1_pool, psum_pool=psum_pool,
            )

            # iteration 0's phase-1 loads, emitted cold; wo weights after
            # them in the queue (first needed only in phase 2)
            pre = _emit_prefetch(env)
            for n in range(HID // SH):
                nc.sync.dma_start(
                    wons[n][:],
                    wo_d[:, :, n * SH:(n + 1) * SH]
                    .rearrange("h p n -> p h n"),
                )
            prev_wo1 = None
            for it in range(unroll):
                pre, prev_wo1 = _emit_iteration(
                    env, it, pre, prefetch_next=(it + 1 < unroll),
                    prev_wo1=prev_wo1)

    nc.compile()
    _BUILT[unroll] = nc
    return nc


def _emit_prefetch(env):
    """Emit the early input loads for one iteration on the SP queue:
    hsT (all 16 k-tiles) and wq heads 0-1.  Returns the tiles for the
    consuming iteration."""
    nc = env["nc"]
    bf16 = env["bf16"]
    hsT = env["hs_pool"].tile([P, KT, S], bf16, tag="hsT", name="hsT_sb")
    for i in range(8):
        nc.sync.dma_start(
            hsT[:, 2 * i:2 * i + 2, :],
            env["hsT_d"][2 * i:2 * i + 2].rearrange("k p s -> p k s"),
        )
    wqs = {}
    for h in range(2):
        wq = env["wq_pool"].tile([P, KT, P], bf16, tag="wq", name="wq_sb")
        nc.sync.dma_start(wq[:], env["wq_d"][h].rearrange("k p m -> p k m"))
        wqs[h] = wq
    return {"hsT": hsT, "wqs": wqs}


def _emit_iteration(env, it, pre, prefetch_next, prev_wo1=None):
    import concourse.tile as tile  # noqa: F401

    nc = env["nc"]
    tc = env["tc"]
    f32, bf16 = env["f32"], env["bf16"]
    mult, add, Exp = env["mult"], env["add"], env["Exp"]
    bass_isa = env["bass_isa"]
    qT, kT, vF = env["qT"], env["kT"], env["vF"]
    attn0 = env["attn0"]
    cs_cc, cs_pm = env["cs_cc"], env["cs_pm"]
    hsT, wqs = pre["hsT"], pre["wqs"]
    wks, wons = env["wks"], env["wons"]
    y_d = env["y_d"]

    ps = env["psum_pool"]

    # ---------------- phase 1: projections + rope ----------------
    with (
        tc.tile_pool(name="rope", bufs=1) as rope_pool,
    ):
        if it == 0:
            # PE warm-up during the initial DMA window; rope tables; caches.
            pw = ps.tile([P, P], f32, tag="av", bufs=1, name="pwarm")
            for i in range(150):
                _mm(nc, pw[:], env["ones128"][:], env["ones128"][:],
                    start=(i == 0), stop=(i == 149), skip_group_check=True)
            wsink = rope_pool.tile([1, 1], f32, tag="wsink", name="wsink")
            nc.vector.tensor_copy(wsink[:], pw[0:1, 0:1])
            nc.sync.dma_start(cs_cc[:], env["cc_d"][:])
            nc.sync.dma_start(cs_pm[:], env["pm_d"][:])
            for g in range(NG):
                nc.sync.dma_start(kT[:, g, 0:C], env["ck_d"][g])
            nc.sync.dma_start(vF[:, 0:C // P, :, :], env["cv_d"][:])

        wv = rope_pool.tile([P, KT, NG * P], bf16, tag="wv", name="wv_sb")
        nc.sync.dma_start(wv[:], env["wv_d"].rearrange("k p n -> p k n"))

        # k projection + rope (new keys go to kT[:, g, C:])
        for g in range(NG):
            pk = ps.tile([P, S], f32, tag="big", bufs=2, name="pk")
            for k in range(KT):
                for n in range(2):
                    _mm(nc, pk[:, n * SH:(n + 1) * SH], wks[g][:, k, :],
                        hsT[:, k, n * SH:(n + 1) * SH],
                        start=(k == 0), stop=(k == KT - 1))
            _rope(nc, rope_pool, f32, pk, kT[:, g, C:C + S], cs_cc, cs_pm,
                  mult)

        # q projection + rope
        for h in range(NH):
            if h in wqs:
                wq = wqs[h]
            else:
                wq = env["wq_pool"].tile([P, KT, P], bf16, tag="wq",
                                         name="wq_sb")
                nc.sync.dma_start(wq[:],
                                  env["wq_d"][h].rearrange("k p m -> p k m"))
            pq = ps.tile([P, S], f32, tag="big", bufs=2, name="pq")
            for k in range(KT):
                for n in range(2):
                    _mm(nc, pq[:, n * SH:(n + 1) * SH], wq[:, k, :],
                        hsT[:, k, n * SH:(n + 1) * SH],
                        start=(k == 0), stop=(k == KT - 1))
            _rope(nc, rope_pool, f32, pq, qT[:, h, :], cs_cc, cs_pm, mult)

        # v projection (natural layout: tokens on partitions)
        for mv in range(S // P):
            pv = ps.tile([P, SH], f32, tag="py", bufs=3, name="pv")
            for k in range(KT):
                _mm(nc, pv[:, 0:NG * P], hsT[:, k, mv * P:(mv + 1) * P],
                    wv[:, k, :], start=(k == 0), stop=(k == KT - 1))
            nc.vector.tensor_copy(vF[:, C // P + mv, :, :], pv[:, 0:NG * P])

    # next iteration's early loads: on the SP queue BEFORE this iteration's
    # y-output triggers, so they stream during this phase 2.
    nxt = _emit_prefetch(env) if prefetch_next else None

    # ---------------- phase 2: attention + output projection ----------------
    with (
        tc.tile_pool(name="probs", bufs=1) as probs_pool,
        tc.tile_pool(name="small", bufs=2) as small_pool,
    ):
        ps2 = ps
        attn1 = env["attn1_pool"].tile([P, NH, SH], bf16, tag="attn1",
                                       name="attn1_sb")

        class WoEmitter:
            """Emits one s-half's output projection as a stream of single
            matmuls so they can be interleaved into the attention pipeline
            as PE filler (the exp chain on ACT otherwise paces PE)."""

            def __init__(self, attn_t, sh):
                self.attn_t = attn_t
                self.sh = sh
                self.jobs = [(n, mt) for n in range(HID // SH)
                             for mt in range(4)]
                self.ji = 0
                self.hi = 0
                self.py = None

            def exhausted(self):
                return self.ji >= len(self.jobs)

            def emit_one(self):
                if self.exhausted():
                    return False
                n, mt = self.jobs[self.ji]
                if self.hi == 0:
                    self.py = ps2.tile([P, SH], f32, tag="py", bufs=3,
                                       name="py")
                h = self.hi
                _mm(nc, self.py[:],
                    self.attn_t[:, h, mt * P:(mt + 1) * P],
                    wons[n][:, h, :], start=(h == 0), stop=(h == NH - 1),
                    skip_group_check=True)
                self.hi += 1
                if self.hi == NH:
                    ysb = small_pool.tile([P, SH], bf16, tag="ysb",
                                          name="ysb")
                    nc.vector.tensor_copy(ysb[:], self.py[:])
                    m = self.sh * 4 + mt
                    nc.sync.dma_start(
                        y_d[m * P:(m + 1) * P, n * SH:(n + 1) * SH], ysb[:],
                    )
                    self.hi = 0
                    self.ji += 1
                return True

            def emit_some(self, k):
                for _ in range(k):
                    if not self.emit_one():
                        return

            def emit_all(self):
                while self.emit_one():
                    pass

        def attention_head(sh, h, attn_t, filler):
            ssl = slice(sh * SH, (sh + 1) * SH)
            g = h // (NH // NG)
            NCG = KC // 2  # chunk groups of 2
            probs = [None] * NCG
            d1 = [None] * NCG
            d2 = [None] * (NCG // 2)
            d3 = [None] * (NCG // 4)
            pav_box = [None]
            den_bc = small_pool.tile([P, SH], f32, tag="denbc", name="den_bc")

            def emit_scores(cg):
                psc = ps2.tile([P, 2, SH], f32, tag="big", bufs=2,
                               name="pscore")
                for j in range(2):
                    c = cg * 2 + j
                    _mm(nc, psc[:, j, :], kT[:, g, c * P:(c + 1) * P],
                        qT[:, h, ssl], start=True, stop=True)
                pt = probs_pool.tile([P, 2, SH], bf16, tag="probs",
                                     bufs=4, name="probs_t")
                nc.scalar.activation(pt[:], psc[:], Exp)
                probs[cg] = pt
                # denominator: DVE pair-sum tree (bf16), one GpSimd
                # partition_all_reduce at the end -> per-s denominator
                # broadcast to every partition; zero PE cycles.
                t1 = probs_pool.tile([P, SH], bf16, tag="d1", bufs=2,
                                     name="d1_t")
                nc.vector.tensor_tensor(t1[:], pt[:, 0, :], pt[:, 1, :], add)
                d1[cg] = t1
                if cg % 2 == 1:
                    t2 = probs_pool.tile([P, SH], bf16, tag="d2", bufs=2,
                                         name="d2_t")
                    nc.vector.tensor_tensor(t2[:], d1[cg - 1][:], t1[:], add)
                    d2[cg // 2] = t2
                if cg % 4 == 3:
                    t3 = probs_pool.tile([P, SH], bf16, tag="d3", bufs=2,
                                         name="d3_t")
                    nc.vector.tensor_tensor(t3[:], d2[cg // 2 - 1][:],
                                            d2[cg // 2][:], add)
                    d3[cg // 4] = t3
                if cg == NCG - 1:
                    t4 = probs_pool.tile([P, SH], bf16, tag="d4", bufs=1,
                                         name="d4_t")
                    nc.vector.tensor_tensor(t4[:], d3[0][:], d3[1][:], add)
                    nc.gpsimd.partition_all_reduce(
                        den_bc[:], t4[:], channels=P,
                        reduce_op=bass_isa.ReduceOp.add)

            def emit_av(cg):
                # lazy: score tiles grab the low PSUM banks, which phase 1's
                # first tiles reuse -- scores free at the last exp, not at
                # the last head's normalize, so the next iteration's
                # projections start ~3.5us earlier.
                if pav_box[0] is None:
                    pav_box[0] = ps2.tile([P, SH], f32, tag="av", bufs=1,
                                          name="pav")
                pav = pav_box[0]
                for j in range(2):
                    c = cg * 2 + j
                    first, last = (c == 0), (c == KC - 1)
                    _mm(nc, pav[:], vF[:, c, g, :], probs[cg][:, j, :],
                        start=first, stop=last, skip_group_check=True)

            # software pipeline: scores 2 groups ahead of AV; up to 2 wo
            # filler matmuls per step keep PE fed while ACT works on exp
            for cg in range(NCG + 2):
                if filler is not None:
                    filler.emit_some(2)
                if cg < NCG:
                    emit_scores(cg)
                if cg >= 2:
                    emit_av(cg - 2)

            # reciprocal of the broadcast denominator, fused into the AV
            # PSUM->SBUF copyback
            rbc = small_pool.tile([P, SH], f32, tag="rbc", name="rbc")
            nc.vector.reciprocal_approx_fast(out=rbc[:], in_=den_bc[:])
            nc.vector.tensor_tensor(attn_t[:, h, :], pav_box[0][:], rbc[:],
                                    mult)

        # s-half 0 heads, filled with the PREVIOUS iteration's s-half-1
        # output projection; s-half 1 heads, filled with this iteration's
        # s-half-0 output projection.
        fill0 = WoEmitter(prev_wo1, 1) if prev_wo1 is not None else None
        for h in range(NH):
            attention_head(0, h, attn0, fill0)
        if fill0 is not None:
            fill0.emit_all()
        fill1 = WoEmitter(attn0, 0)
        for h in range(NH):
            attention_head(1, h, attn1, fill1)
        fill1.emit_all()
        if not prefetch_next:
            # last iteration: its s-half-1 wo has no later home
            WoEmitter(attn1, 1).emit_all()

        # next iteration's kv-cache loads ride the ACT queue: ACT reaches
        # them right after its last exp, and kT/vF are free then too.
        if prefetch_next:
            for g in range(NG):
                nc.scalar.dma_start(kT[:, g, 0:C], env["ck_d"][g])
            nc.scalar.dma_start(vF[:, 0:C // P, :, :], env["cv_d"][:])

    return nxt, attn1


def prep_inputs(hidden_states, freqs_cos, freqs_sin, cache_k, cache_v,
                wq, wk, wv, wo):
    """Shard + pre-transpose the full inputs into 8 per-core input maps."""
    f = np.float32
    b16 = ml_dtypes.bfloat16
    scale = np.float32(1.0 / np.sqrt(D))
    wq_p = (wq.astype(f).reshape(H, D, HID)[:, _PERM, :] * scale)
    wk_p = wk.astype(f).reshape(G, D, HID)[:, _PERM, :]
    wv_r = wv.astype(f).reshape(G, D, HID)

    cc = freqs_cos.astype(f).T          # [64, S]
    ss = freqs_sin.astype(f).T
    cs_cc = np.ascontiguousarray(np.concatenate([cc, cc], axis=0))
    cs_pm = np.ascontiguousarray(np.concatenate([ss, -ss], axis=0))

    in_maps = []
    for b in range(B):
        hsT = np.ascontiguousarray(
            hidden_states[b].astype(f).T.reshape(KT, P, S)).astype(b16)
        for hg in range(2):
            hs_q = slice(hg * NH, (hg + 1) * NH)
            hs_kv = slice(hg * NG, (hg + 1) * NG)
            wqT = wq_p[hs_q].reshape(NH * D, HID).T          # [HID, 1024]
            wqT_t = np.ascontiguousarray(
                wqT.reshape(KT, P, NH, P).transpose(2, 0, 1, 3)).astype(b16)
            wkT = wk_p[hs_kv].reshape(NG * D, HID).T         # [HID, 256]
            wkT_t = np.ascontiguousarray(
                wkT.reshape(KT, P, NG, P).transpose(2, 0, 1, 3)).astype(b16)
            wvT = wv_r[hs_kv].reshape(NG * D, HID).T         # [HID, 256]
            wvT_t = np.ascontiguousarray(
                wvT.reshape(KT, P, NG * P)).astype(b16)
            woT = np.ascontiguousarray(
                wo.astype(f)[:, hg * NH * D:(hg + 1) * NH * D].T
                .reshape(NH, P, HID)).astype(b16)
            ckT = np.ascontiguousarray(
                cache_k[b].astype(f)[:, hs_kv][:, :, _PERM]
                .transpose(1, 2, 0)).astype(b16)
            cvP = np.ascontiguousarray(
                cache_v[b].astype(f)[:, hs_kv]
                .reshape(C // P, P, NG, P).transpose(1, 0, 2, 3)).astype(b16)
            in_maps.append({
                "hsT": hsT, "wqT": wqT_t, "wkT": wkT_t, "wvT": wvT_t,
                "woT": woT, "ckT": ckT, "cvP": cvP,
                "cs_cc": cs_cc, "cs_pm": cs_pm,
            })
    return in_maps


def gather_output(results):
    """Sum the 2 TP partials per batch -> full [B, S, HID] output."""
    out = np.empty((B, S, HID), np.float32)
    for b in range(B):
        out[b] = results[2 * b]["y"] + results[2 * b + 1]["y"]
    return out


def kernel(hidden_states, freqs_cos, freqs_sin, attention_mask,
           cache_k, cache_v, wq, wk, wv, wo):
    # attention_mask is all-zeros by construction (see spec) - unused.
    from concourse.bass_utils import run_bass_kernel_spmd

    nc = build_bass(unroll=1)
    in_maps = prep_inputs(
        np.asarray(hidden_states), np.asarray(freqs_cos), np.asarray(freqs_sin),
        np.asarray(cache_k), np.asarray(cache_v),
        np.asarray(wq), np.asarray(wk), np.asarray(wv), np.asarray(wo))
    res = run_bass_kernel_spmd(nc, in_maps, core_ids=list(range(N_CORES)))
    return gather_output(res.results)


# revision 28
# speedup vs baseline: 1.2188x; 1.0547x over previous
"""BrahmaAttention (GQA prefill with KV cache) on 8 Trainium2 NeuronCores.

Problem: B=4, S=1024, C=1024 (cache), H=16 q-heads, G=4 kv-heads, D=128.
    q = hs @ wq.T ; k = hs @ wk.T ; v = hs @ wv.T
    rope(q, k) (interleaved pairs, positions C..C+S)
    k_full/v_full = concat(cache, new)           # K = 2048 keys
    out = softmax(q k^T / sqrt(D)) @ v_full @ wo.T
(attention_mask is all-zeros by construction - full attention, no masking.)

Sharding: 4-way data parallel over batch x 2-way tensor parallel over heads.
core (b, hg) handles batch b, q-heads hg*8..hg*8+8, kv-heads hg*2..hg*2+2 and
computes a partial output projection over its 1024 hidden columns; the host
sums the two partials per batch (the TP all-reduce done on host at gather).

Host-side prep folded into the shards:
  - 1/sqrt(D) folded into wq.
  - RoPE even/odd interleave permuted to [evens|odds] via wq/wk row
    permutation and cache_k last-dim permutation, so on-chip RoPE is
    half-tile elementwise ops (partitions 0-63 = even, 64-127 = odd lanes).
  - All projection weights pre-transposed/tiled so every DMA is contiguous,
    and shipped in bf16 (PE streams bf16 at the same 1 col/cycle as f32r,
    so bf16 costs nothing on PE and halves DMA + SBUF).

On-chip structure per core (all matmul operands bf16, PSUM f32):
  phase 1: q/k/v projections (PE) + rope (DVE + ACT-copy swap)
  phase 2: scoresT = kT.T @ qT -> PSUM -> exp (ACT) -> probs bf16
           softmax denominator: DVE pair-sum tree over the 16 key chunks,
           then ONE GpSimd partition_all_reduce => per-s denominator already
           broadcast to all partitions (zero PE cycles, zero bcast matmul)
           AV accumulated on PE; normalize fused into the PSUM->SBUF copy
           wo projection; s-half 0's wo tiles interleaved between s-half 1's
           heads to fill PE while ACT paces the exp chain
  cross-iteration software pipeline: hsT/wk/wq tiles double-buffered in
  always-open pools; the NEXT iteration's input DMAs are emitted on the SP
  queue before this iteration's output DMAs, so they stream during phase 2
  instead of serializing at the iteration boundary.  kv-cache loads ride the
  ACT queue (free after its last exp).
"""

import numpy as np
import ml_dtypes

B, S, C, H, G, D = 4, 1024, 1024, 16, 4, 128
HID = H * D
P = 128
NH, NG = 8, 2          # per-core q heads / kv heads
KC = (C + S) // P      # 16 key chunks
KT = 16                # hid contraction tiles
SH = 512               # s-half (PSUM bank free size)
N_CORES = 8

_PERM = np.concatenate([np.arange(0, D, 2), np.arange(1, D, 2)])

_BUILT = {}


def _mm(nc, out, lhsT, rhs, **kw):
    nc.tensor.matmul(out, lhsT, rhs, **kw)


def _rope(nc, pool, f32, psum_in, out_ap, cs_cc, cs_pm, mult):
    """out = psum_in*[cos;cos] + swap_halves(psum_in*[sin;-sin]).

    psum_in is the raw projected [128, S] tile with evens on partitions 0-63
    and odds on 64-127; out gets the roped value in the same layout.
    """
    import concourse.mybir as mybir

    a = pool.tile([P, S], f32, tag="ropeA", name="ropeA")
    b = pool.tile([P, S], f32, tag="ropeB", name="ropeB")
    s = pool.tile([P, S], f32, tag="ropeS", name="ropeS")
    nc.vector.tensor_tensor(a[:], psum_in[:], cs_cc[:], mult)
    nc.vector.tensor_tensor(b[:], psum_in[:], cs_pm[:], mult)
    # swap halves on the scalar engine (idle during phase 1)
    nc.scalar.copy(s[0:64, :], b[64:128, :])
    nc.scalar.copy(s[64:128, :], b[0:64, :])
    nc.vector.tensor_tensor(out_ap, a[:], s[:], mybir.AluOpType.add)


def build_bass(unroll=1):
    """Build + compile the per-core Bass program (identical on all cores)."""
    if unroll in _BUILT:
        return _BUILT[unroll]

    import concourse.mybir as mybir
    import concourse.tile as tile
    import concourse.bass_isa as bass_isa
    from concourse import bacc

    f32 = mybir.dt.float32
    f32r = mybir.dt.float32r
    bf16 = mybir.dt.bfloat16
    mult = mybir.AluOpType.mult
    add = mybir.AluOpType.add
    Exp = mybir.ActivationFunctionType.Exp

    nc = bacc.Bacc("TRN2", target_bir_lowering=False, debug=False)

    hsT_d = nc.dram_tensor("hsT", [KT, P, S], bf16, kind="ExternalInput")
    wq_d = nc.dram_tensor("wqT", [NH, KT, P, P], bf16, kind="ExternalInput")
    wk_d = nc.dram_tensor("wkT", [NG, KT, P, P], bf16, kind="ExternalInput")
    wv_d = nc.dram_tensor("wvT", [KT, P, NG * P], bf16, kind="ExternalInput")
    wo_d = nc.dram_tensor("woT", [NH, P, HID], bf16, kind="ExternalInput")
    ck_d = nc.dram_tensor("ckT", [NG, P, C], bf16, kind="ExternalInput")
    cv_d = nc.dram_tensor("cvP", [P, C // P, NG, P], bf16, kind="ExternalInput")
    cc_d = nc.dram_tensor("cs_cc", [P, S], f32, kind="ExternalInput")
    pm_d = nc.dram_tensor("cs_pm", [P, S], f32, kind="ExternalInput")
    y_d = nc.dram_tensor("y", [S, HID], bf16, kind="ExternalOutput")

    with tile.TileContext(nc) as tc:
        with (
            tc.tile_pool(name="const", bufs=1) as const,
            tc.tile_pool(name="hs", bufs=2) as hs_pool,
            tc.tile_pool(name="wq_pool", bufs=2) as wq_pool,
            tc.tile_pool(name="wk_pool", bufs=2) as wk_pool,
            tc.tile_pool(name="a1", bufs=2) as a1_pool,
            tc.tile_pool(name="persist", bufs=1) as persist,
            tc.tile_pool(name="psum", bufs=1, space="PSUM") as psum_pool,
        ):
            ones_f = const.tile([P, P], f32, name="ones_f")
            nc.any.memset(ones_f[:], 1.0)
            ones128 = const.tile([P, P], f32r, name="ones128")
            nc.vector.tensor_copy(ones128[:], ones_f[:])
            cs_cc = const.tile([P, S], f32, name="cs_cc")
            cs_pm = const.tile([P, S], f32, name="cs_pm")

            qT = persist.tile([P, NH, S], bf16, name="qT")
            kT = persist.tile([P, NG, C + S], bf16, name="kT")
            vF = persist.tile([P, KC, NG, P], bf16, name="vF")
            attn0 = persist.tile([P, NH, SH], bf16, name="attn0")

            # weights identical across unrolled iterations: wk and wo are
            # loaded once up front and never reloaded.
            wks = []
            for g in range(NG):
                wk = wk_pool.tile([P, KT, P], bf16, tag="wk", name="wk_sb")
                nc.sync.dma_start(wk[:],
                                  wk_d[g].rearrange("k p m -> p k m"))
                wks.append(wk)
            wons = [persist.tile([P, NH, SH], bf16, tag=f"won{n}",
                                 name="won") for n in range(HID // SH)]

            env = dict(
                nc=nc, tc=tc, f32=f32, f32r=f32r, bf16=bf16, mult=mult,
                add=add, Exp=Exp, bass_isa=bass_isa,
                hsT_d=hsT_d, wq_d=wq_d, wk_d=wk_d, wv_d=wv_d, wo_d=wo_d,
                ck_d=ck_d, cv_d=cv_d, cc_d=cc_d, pm_d=pm_d, y_d=y_d,
                hs_pool=hs_pool, wq_pool=wq_pool, wk_pool=wk_pool,
                qT=qT, kT=kT, vF=vF, attn0=attn0,
                wks=wks, wons=wons,
                ones128=ones128, cs_cc=cs_cc, cs_pm=cs_pm,
                attn1_pool=a1_pool, psum_pool=psum_pool,
            )

            # iteration 0's phase-1 loads, emitted cold; wo weights after
            # them in the queue (first needed only in phase 2)
            pre = _emit_prefetch(env)
            for n in range(HID // SH):
                nc.sync.dma_start(
                    wons[n][:],
                    wo_d[:, :, n * SH:(n + 1) * SH]
                    .rearrange("h p n -> p h n"),
                )
            prev_wo1 = None
            for it in range(unroll):
                pre, prev_wo1 = _emit_iteration(
                    env, it, pre, prefetch_next=(it + 1 < unroll),
                    prev_wo1=prev_wo1)

    nc.compile()
    _BUILT[unroll] = nc
    return nc


def _emit_prefetch(env):
    """Emit the early input loads for one iteration on the SP queue:
    hsT (all 16 k-tiles) and wq heads 0-1.  Returns the tiles for the
    consuming iteration."""
    nc = env["nc"]
    bf16 = env["bf16"]
    hsT = env["hs_pool"].tile([P, KT, S], bf16, tag="hsT", name="hsT_sb")
    for i in range(8):
        nc.sync.dma_start(
            hsT[:, 2 * i:2 * i + 2, :],
            env["hsT_d"][2 * i:2 * i + 2].rearrange("k p s -> p k s"),
        )
    wqs = {}
    for h in range(2):
        wq = env["wq_pool"].tile([P, KT, P], bf16, tag="wq", name="wq_sb")
        nc.sync.dma_start(wq[:], env["wq_d"][h].rearrange("k p m -> p k m"))
        wqs[h] = wq
    return {"hsT": hsT, "wqs": wqs}


def _emit_iteration(env, it, pre, prefetch_next, prev_wo1=None):
    import concourse.tile as tile  # noqa: F401

    nc = env["nc"]
    tc = env["tc"]
    f32, bf16 = env["f32"], env["bf16"]
    mult, add, Exp = env["mult"], env["add"], env["Exp"]
    bass_isa = env["bass_isa"]
    qT, kT, vF = env["qT"], env["kT"], env["vF"]
    attn0 = env["attn0"]
    cs_cc, cs_pm = env["cs_cc"], env["cs_pm"]
    hsT, wqs = pre["hsT"], pre["wqs"]
    wks, wons = env["wks"], env["wons"]
    y_d = env["y_d"]

    ps = env["psum_pool"]

    # ---------------- phase 1: projections + rope ----------------
    with (
        tc.tile_pool(name="rope", bufs=1) as rope_pool,
    ):
        if it == 0:
            # PE warm-up during the initial DMA window; rope tables; caches.
            pw = ps.tile([P, P], f32, tag="av", bufs=2, name="pwarm")
            for i in range(150):
                _mm(nc, pw[:], env["ones128"][:], env["ones128"][:],
                    start=(i == 0), stop=(i == 149), skip_group_check=True)
            wsink = rope_pool.tile([1, 1], f32, tag="wsink", name="wsink")
            nc.vector.tensor_copy(wsink[:], pw[0:1, 0:1])
            nc.sync.dma_start(cs_cc[:], env["cc_d"][:])
            nc.sync.dma_start(cs_pm[:], env["pm_d"][:])
            for g in range(NG):
                nc.sync.dma_start(kT[:, g, 0:C], env["ck_d"][g])
            nc.sync.dma_start(vF[:, 0:C // P, :, :], env["cv_d"][:])

        wv = rope_pool.tile([P, KT, NG * P], bf16, tag="wv", name="wv_sb")
        nc.sync.dma_start(wv[:], env["wv_d"].rearrange("k p n -> p k n"))

        # k projection + rope (new keys go to kT[:, g, C:])
        for g in range(NG):
            pk = ps.tile([P, S], f32, tag="big", bufs=2, name="pk")
            for k in range(KT):
                for n in range(2):
                    _mm(nc, pk[:, n * SH:(n + 1) * SH], wks[g][:, k, :],
                        hsT[:, k, n * SH:(n + 1) * SH],
                        start=(k == 0), stop=(k == KT - 1))
            _rope(nc, rope_pool, f32, pk, kT[:, g, C:C + S], cs_cc, cs_pm,
                  mult)

        # q projection + rope
        for h in range(NH):
            if h in wqs:
                wq = wqs[h]
            else:
                wq = env["wq_pool"].tile([P, KT, P], bf16, tag="wq",
                                         name="wq_sb")
                nc.sync.dma_start(wq[:],
                                  env["wq_d"][h].rearrange("k p m -> p k m"))
            pq = ps.tile([P, S], f32, tag="big", bufs=2, name="pq")
            for k in range(KT):
                for n in range(2):
                    _mm(nc, pq[:, n * SH:(n + 1) * SH], wq[:, k, :],
                        hsT[:, k, n * SH:(n + 1) * SH],
                        start=(k == 0), stop=(k == KT - 1))
            _rope(nc, rope_pool, f32, pq, qT[:, h, :], cs_cc, cs_pm, mult)

        # v projection (natural layout: tokens on partitions)
        for mv in range(S // P):
            pv = ps.tile([P, SH], f32, tag="py", bufs=2, name="pv")
            for k in range(KT):
                _mm(nc, pv[:, 0:NG * P], hsT[:, k, mv * P:(mv + 1) * P],
                    wv[:, k, :], start=(k == 0), stop=(k == KT - 1))
            nc.vector.tensor_copy(vF[:, C // P + mv, :, :], pv[:, 0:NG * P])

    # next iteration's early loads: on the SP queue BEFORE this iteration's
    # y-output triggers, so they stream during this phase 2.
    nxt = _emit_prefetch(env) if prefetch_next else None

    # ---------------- phase 2: attention + output projection ----------------
    with (
        tc.tile_pool(name="probs", bufs=1) as probs_pool,
        tc.tile_pool(name="small", bufs=2) as small_pool,
    ):
        ps2 = ps
        attn1 = env["attn1_pool"].tile([P, NH, SH], bf16, tag="attn1",
                                       name="attn1_sb")

        class WoEmitter:
            """Emits one s-half's output projection as a stream of single
            matmuls so they can be interleaved into the attention pipeline
            as PE filler (the exp chain on ACT otherwise paces PE)."""

            def __init__(self, attn_t, sh):
                self.attn_t = attn_t
                self.sh = sh
                self.jobs = [(n, mt) for n in range(HID // SH)
                             for mt in range(4)]
                self.ji = 0
                self.hi = 0
                self.py = None

            def exhausted(self):
                return self.ji >= len(self.jobs)

            def emit_one(self):
                if self.exhausted():
                    return False
                n, mt = self.jobs[self.ji]
                if self.hi == 0:
                    self.py = ps2.tile([P, SH], f32, tag="py", bufs=2,
                                       name="py")
                h = self.hi
                _mm(nc, self.py[:],
                    self.attn_t[:, h, mt * P:(mt + 1) * P],
                    wons[n][:, h, :], start=(h == 0), stop=(h == NH - 1),
                    skip_group_check=True)
                self.hi += 1
                if self.hi == NH:
                    ysb = small_pool.tile([P, SH], bf16, tag="ysb",
                                          name="ysb")
                    nc.vector.tensor_copy(ysb[:], self.py[:])
                    m = self.sh * 4 + mt
                    nc.sync.dma_start(
                        y_d[m * P:(m + 1) * P, n * SH:(n + 1) * SH], ysb[:],
                    )
                    self.hi = 0
                    self.ji += 1
                return True

            def emit_some(self, k):
                for _ in range(k):
                    if not self.emit_one():
                        return

            def emit_all(self):
                while self.emit_one():
                    pass

        def attention_head(sh, h, attn_t, filler):
            ssl = slice(sh * SH, (sh + 1) * SH)
            g = h // (NH // NG)
            NCG = KC // 2  # chunk groups of 2
            probs = [None] * NCG
            d1 = [None] * NCG
            d2 = [None] * (NCG // 2)
            d3 = [None] * (NCG // 4)
            pav_box = [None]
            den_bc = small_pool.tile([P, SH], f32, tag="denbc", name="den_bc")

            def emit_scores(cg):
                psc = ps2.tile([P, 2, SH], f32, tag="big", bufs=2,
                               name="pscore")
                for j in range(2):
                    c = cg * 2 + j
                    _mm(nc, psc[:, j, :], kT[:, g, c * P:(c + 1) * P],
                        qT[:, h, ssl], start=True, stop=True)
                pt = probs_pool.tile([P, 2, SH], bf16, tag="probs",
                                     bufs=4, name="probs_t")
                nc.scalar.activation(pt[:], psc[:], Exp)
                probs[cg] = pt
                # denominator: DVE pair-sum tree (bf16), one GpSimd
                # partition_all_reduce at the end -> per-s denominator
                # broadcast to every partition; zero PE cycles.
                t1 = probs_pool.tile([P, SH], bf16, tag="d1", bufs=2,
                                     name="d1_t")
                nc.vector.tensor_tensor(t1[:], pt[:, 0, :], pt[:, 1, :], add)
                d1[cg] = t1
                if cg % 2 == 1:
                    t2 = probs_pool.tile([P, SH], bf16, tag="d2", bufs=2,
                                         name="d2_t")
                    nc.vector.tensor_tensor(t2[:], d1[cg - 1][:], t1[:], add)
                    d2[cg // 2] = t2
                if cg % 4 == 3:
                    t3 = probs_pool.tile([P, SH], bf16, tag="d3", bufs=2,
                                         name="d3_t")
                    nc.vector.tensor_tensor(t3[:], d2[cg // 2 - 1][:],
                                            d2[cg // 2][:], add)
                    d3[cg // 4] = t3
                if cg == NCG - 1:
                    t4 = probs_pool.tile([P, SH], bf16, tag="d4", bufs=1,
                                         name="d4_t")
                    nc.vector.tensor_tensor(t4[:], d3[0][:], d3[1][:], add)
                    nc.gpsimd.partition_all_reduce(
                        den_bc[:], t4[:], channels=P,
                        reduce_op=bass_isa.ReduceOp.add)

            def emit_av(cg):
                # lazy: score tiles grab the low PSUM banks, which phase 1's
                # first tiles reuse -- scores free at the last exp, not at
                # the last head's normalize, so the next iteration's
                # projections start ~3.5us earlier.
                if pav_box[0] is None:
                    pav_box[0] = ps2.tile([P, SH], f32, tag="av", bufs=2,
                                          name="pav")
                pav = pav_box[0]
                for j in range(2):
                    c = cg * 2 + j
                    first, last = (c == 0), (c == KC - 1)
                    _mm(nc, pav[:], vF[:, c, g, :], probs[cg][:, j, :],
                        start=first, stop=last, skip_group_check=True)

            # software pipeline: scores 2 groups ahead of AV; up to 2 wo
            # filler matmuls per step keep PE fed while ACT works on exp
            for cg in range(NCG + 2):
                if filler is not None:
                    filler.emit_some(2)
                if cg < NCG:
                    emit_scores(cg)
                if cg >= 2:
                    emit_av(cg - 2)

            # reciprocal of the broadcast denominator, fused into the AV
            # PSUM->SBUF copyback
            rbc = small_pool.tile([P, SH], f32, tag="rbc", name="rbc")
            nc.vector.reciprocal_approx_fast(out=rbc[:], in_=den_bc[:])
            nc.vector.tensor_tensor(attn_t[:, h, :], pav_box[0][:], rbc[:],
                                    mult)

        # s-half 0 heads, filled with the PREVIOUS iteration's s-half-1
        # output projection; s-half 1 heads, filled with this iteration's
        # s-half-0 output projection.
        fill0 = WoEmitter(prev_wo1, 1) if prev_wo1 is not None else None
        for h in range(NH):
            attention_head(0, h, attn0, fill0)
        if fill0 is not None:
            fill0.emit_all()
        fill1 = WoEmitter(attn0, 0)
        for h in range(NH):
            attention_head(1, h, attn1, fill1)
        fill1.emit_all()
        if not prefetch_next:
            # last iteration: its s-half-1 wo has no later home
            WoEmitter(attn1, 1).emit_all()

        # next iteration's kv-cache loads ride the ACT queue: ACT reaches
        # them right after its last exp, and kT/vF are free then too.
        if prefetch_next:
            for g in range(NG):
                nc.scalar.dma_start(kT[:, g, 0:C], env["ck_d"][g])
            nc.scalar.dma_start(vF[:, 0:C // P, :, :], env["cv_d"][:])

    return nxt, attn1


def prep_inputs(hidden_states, freqs_cos, freqs_sin, cache_k, cache_v,
                wq, wk, wv, wo):
    """Shard + pre-transpose the full inputs into 8 per-core input maps."""
    f = np.float32
    b16 = ml_dtypes.bfloat16
    scale = np.float32(1.0 / np.sqrt(D))
    wq_p = (wq.astype(f).reshape(H, D, HID)[:, _PERM, :] * scale)
    wk_p = wk.astype(f).reshape(G, D, HID)[:, _PERM, :]
    wv_r = wv.astype(f).reshape(G, D, HID)

    cc = freqs_cos.astype(f).T          # [64, S]
    ss = freqs_sin.astype(f).T
    cs_cc = np.ascontiguousarray(np.concatenate([cc, cc], axis=0))
    cs_pm = np.ascontiguousarray(np.concatenate([ss, -ss], axis=0))

    in_maps = []
    for b in range(B):
        hsT = np.ascontiguousarray(
            hidden_states[b].astype(f).T.reshape(KT, P, S)).astype(b16)
        for hg in range(2):
            hs_q = slice(hg * NH, (hg + 1) * NH)
            hs_kv = slice(hg * NG, (hg + 1) * NG)
            wqT = wq_p[hs_q].reshape(NH * D, HID).T          # [HID, 1024]
            wqT_t = np.ascontiguousarray(
                wqT.reshape(KT, P, NH, P).transpose(2, 0, 1, 3)).astype(b16)
            wkT = wk_p[hs_kv].reshape(NG * D, HID).T         # [HID, 256]
            wkT_t = np.ascontiguousarray(
                wkT.reshape(KT, P, NG, P).transpose(2, 0, 1, 3)).astype(b16)
            wvT = wv_r[hs_kv].reshape(NG * D, HID).T         # [HID, 256]
            wvT_t = np.ascontiguousarray(
                wvT.reshape(KT, P, NG * P)).astype(b16)
            woT = np.ascontiguousarray(
                wo.astype(f)[:, hg * NH * D:(hg + 1) * NH * D].T
                .reshape(NH, P, HID)).astype(b16)
            ckT = np.ascontiguousarray(
                cache_k[b].astype(f)[:, hs_kv][:, :, _PERM]
                .transpose(1, 2, 0)).astype(b16)
            cvP = np.ascontiguousarray(
                cache_v[b].astype(f)[:, hs_kv]
                .reshape(C // P, P, NG, P).transpose(1, 0, 2, 3)).astype(b16)
            in_maps.append({
                "hsT": hsT, "wqT": wqT_t, "wkT": wkT_t, "wvT": wvT_t,
                "woT": woT, "ckT": ckT, "cvP": cvP,
                "cs_cc": cs_cc, "cs_pm": cs_pm,
            })
    return in_maps


def gather_output(results):
    """Sum the 2 TP partials per batch -> full [B, S, HID] output."""
    out = np.empty((B, S, HID), np.float32)
    for b in range(B):
        out[b] = results[2 * b]["y"] + results[2 * b + 1]["y"]
    return out


def kernel(hidden_states, freqs_cos, freqs_sin, attention_mask,
           cache_k, cache_v, wq, wk, wv, wo):
    # attention_mask is all-zeros by construction (see spec) - unused.
    from concourse.bass_utils import run_bass_kernel_spmd

    nc = build_bass(unroll=1)
    in_maps = prep_inputs(
        np.asarray(hidden_states), np.asarray(freqs_cos), np.asarray(freqs_sin),
        np.asarray(cache_k), np.asarray(cache_v),
        np.asarray(wq), np.asarray(wk), np.asarray(wv), np.asarray(wo))
    res = run_bass_kernel_spmd(nc, in_maps, core_ids=list(range(N_CORES)))
    return gather_output(res.results)


# revision 30
# speedup vs baseline: 1.2621x; 1.0355x over previous
"""BrahmaAttention (GQA prefill with KV cache) on 8 Trainium2 NeuronCores.

Problem: B=4, S=1024, C=1024 (cache), H=16 q-heads, G=4 kv-heads, D=128.
    q = hs @ wq.T ; k = hs @ wk.T ; v = hs @ wv.T
    rope(q, k) (interleaved pairs, positions C..C+S)
    k_full/v_full = concat(cache, new)           # K = 2048 keys
    out = softmax(q k^T / sqrt(D)) @ v_full @ wo.T
(attention_mask is all-zeros by construction - full attention, no masking.)

Sharding: 4-way data parallel over batch x 2-way tensor parallel over heads.
core (b, hg) handles batch b, q-heads hg*8..hg*8+8, kv-heads hg*2..hg*2+2 and
computes a partial output projection over its 1024 hidden columns; the host
sums the two partials per batch (the TP all-reduce done on host at gather).

Host-side prep folded into the shards:
  - 1/sqrt(D) folded into wq.
  - RoPE even/odd interleave permuted to [evens|odds] via wq/wk row
    permutation and cache_k last-dim permutation, so on-chip RoPE is
    half-tile elementwise ops (partitions 0-63 = even, 64-127 = odd lanes).
  - All projection weights pre-transposed/tiled so every DMA is contiguous,
    and shipped in bf16 (PE streams bf16 at the same 1 col/cycle as f32r,
    so bf16 costs nothing on PE and halves DMA + SBUF).

On-chip structure per core (all matmul operands bf16, PSUM f32):
  phase 1: q/k/v projections (PE) + rope (DVE + ACT-copy swap)
  phase 2: scoresT = kT.T @ qT -> PSUM -> exp (ACT) -> probs bf16
           softmax denominator: DVE pair-sum tree over the 16 key chunks,
           then ONE GpSimd partition_all_reduce => per-s denominator already
           broadcast to all partitions (zero PE cycles, zero bcast matmul)
           AV accumulated on PE; normalize fused into the PSUM->SBUF copy
           wo projection; s-half 0's wo tiles interleaved between s-half 1's
           heads to fill PE while ACT paces the exp chain
  cross-iteration software pipeline: hsT/wk/wq tiles double-buffered in
  always-open pools; the NEXT iteration's input DMAs are emitted on the SP
  queue before this iteration's output DMAs, so they stream during phase 2
  instead of serializing at the iteration boundary.  kv-cache loads ride the
  ACT queue (free after its last exp).
"""

import numpy as np
import ml_dtypes

B, S, C, H, G, D = 4, 1024, 1024, 16, 4, 128
HID = H * D
P = 128
NH, NG = 8, 2          # per-core q heads / kv heads
KC = (C + S) // P      # 16 key chunks
KT = 16                # hid contraction tiles
SH = 512               # s-half (PSUM bank free size)
N_CORES = 8

_PERM = np.concatenate([np.arange(0, D, 2), np.arange(1, D, 2)])

_BUILT = {}


def _mm(nc, out, lhsT, rhs, **kw):
    nc.tensor.matmul(out, lhsT, rhs, **kw)


def _rope(nc, pool, f32, psum_in, out_ap, cs_cc, cs_pm, mult):
    """out = psum_in*[cos;cos] + swap_halves(psum_in*[sin;-sin]).

    psum_in is the raw projected [128, S] tile with evens on partitions 0-63
    and odds on 64-127; out gets the roped value in the same layout.
    """
    import concourse.mybir as mybir

    a = pool.tile([P, S], f32, tag="ropeA", name="ropeA")
    b = pool.tile([P, S], f32, tag="ropeB", name="ropeB")
    s = pool.tile([P, S], f32, tag="ropeS", name="ropeS")
    nc.vector.tensor_tensor(a[:], psum_in[:], cs_cc[:], mult)
    nc.vector.tensor_tensor(b[:], psum_in[:], cs_pm[:], mult)
    # swap halves on the scalar engine (idle during phase 1)
    nc.scalar.copy(s[0:64, :], b[64:128, :])
    nc.scalar.copy(s[64:128, :], b[0:64, :])
    nc.vector.tensor_tensor(out_ap, a[:], s[:], mybir.AluOpType.add)


def build_bass(unroll=1):
    """Build + compile the per-core Bass program (identical on all cores)."""
    if unroll in _BUILT:
        return _BUILT[unroll]

    import concourse.mybir as mybir
    import concourse.tile as tile
    import concourse.bass_isa as bass_isa
    from concourse import bacc

    f32 = mybir.dt.float32
    f32r = mybir.dt.float32r
    bf16 = mybir.dt.bfloat16
    mult = mybir.AluOpType.mult
    add = mybir.AluOpType.add
    Exp = mybir.ActivationFunctionType.Exp

    nc = bacc.Bacc("TRN2", target_bir_lowering=False, debug=False)

    hsT_d = nc.dram_tensor("hsT", [KT, P, S], bf16, kind="ExternalInput")
    wq_d = nc.dram_tensor("wqT", [NH, KT, P, P], bf16, kind="ExternalInput")
    wk_d = nc.dram_tensor("wkT", [NG, KT, P, P], bf16, kind="ExternalInput")
    wv_d = nc.dram_tensor("wvT", [KT, P, NG * P], bf16, kind="ExternalInput")
    wo_d = nc.dram_tensor("woT", [NH, P, HID], bf16, kind="ExternalInput")
    ck_d = nc.dram_tensor("ckT", [NG, P, C], bf16, kind="ExternalInput")
    cv_d = nc.dram_tensor("cvP", [P, C // P, NG, P], bf16, kind="ExternalInput")
    cc_d = nc.dram_tensor("cs_cc", [P, S], f32, kind="ExternalInput")
    pm_d = nc.dram_tensor("cs_pm", [P, S], f32, kind="ExternalInput")
    y_d = nc.dram_tensor("y", [S, HID], bf16, kind="ExternalOutput")

    with tile.TileContext(nc) as tc:
        with (
            tc.tile_pool(name="const", bufs=1) as const,
            tc.tile_pool(name="hs", bufs=2) as hs_pool,
            tc.tile_pool(name="wq_pool", bufs=2) as wq_pool,
            tc.tile_pool(name="wk_pool", bufs=2) as wk_pool,
            tc.tile_pool(name="a1", bufs=2) as a1_pool,
            tc.tile_pool(name="persist", bufs=1) as persist,
            tc.tile_pool(name="psum", bufs=1, space="PSUM") as psum_pool,
        ):
            ones_f = const.tile([P, P], f32, name="ones_f")
            nc.any.memset(ones_f[:], 1.0)
            ones128 = const.tile([P, P], f32r, name="ones128")
            nc.vector.tensor_copy(ones128[:], ones_f[:])
            cs_cc = const.tile([P, S], f32, name="cs_cc")
            cs_pm = const.tile([P, S], f32, name="cs_pm")

            qT = persist.tile([P, NH, S], bf16, name="qT")
            kT = persist.tile([P, NG, C + S], bf16, name="kT")
            vF = persist.tile([P, KC, NG, P], bf16, name="vF")
            attn0 = persist.tile([P, NH, SH], bf16, name="attn0")

            # weights identical across unrolled iterations: wk and wo are
            # loaded once up front and never reloaded.
            wks = []
            for g in range(NG):
                wk = wk_pool.tile([P, KT, P], bf16, tag="wk", name="wk_sb")
                nc.sync.dma_start(wk[:],
                                  wk_d[g].rearrange("k p m -> p k m"))
                wks.append(wk)
            wons = [persist.tile([P, NH, SH], bf16, tag=f"won{n}",
                                 name="won") for n in range(HID // SH)]

            env = dict(
                nc=nc, tc=tc, f32=f32, f32r=f32r, bf16=bf16, mult=mult,
                add=add, Exp=Exp, bass_isa=bass_isa,
                hsT_d=hsT_d, wq_d=wq_d, wk_d=wk_d, wv_d=wv_d, wo_d=wo_d,
                ck_d=ck_d, cv_d=cv_d, cc_d=cc_d, pm_d=pm_d, y_d=y_d,
                hs_pool=hs_pool, wq_pool=wq_pool, wk_pool=wk_pool,
                qT=qT, kT=kT, vF=vF, attn0=attn0,
                wks=wks, wons=wons,
                ones128=ones128, cs_cc=cs_cc, cs_pm=cs_pm,
                attn1_pool=a1_pool, psum_pool=psum_pool,
            )

            # iteration 0's phase-1 loads, emitted cold; wo weights after
            # them in the queue (first needed only in phase 2)
            pre = _emit_prefetch(env)
            for n in range(HID // SH):
                nc.sync.dma_start(
                    wons[n][:],
                    wo_d[:, :, n * SH:(n + 1) * SH]
                    .rearrange("h p n -> p h n"),
                )
            prev_wo1 = None
            for it in range(unroll):
                pre, prev_wo1 = _emit_iteration(
                    env, it, pre, prefetch_next=(it + 1 < unroll),
                    prev_wo1=prev_wo1)

    nc.compile()
    _BUILT[unroll] = nc
    return nc


def _emit_prefetch(env):
    """Emit the early input loads for one iteration on the SP queue:
    hsT (all 16 k-tiles) and wq heads 0-1.  Returns the tiles for the
    consuming iteration."""
    nc = env["nc"]
    bf16 = env["bf16"]
    hsT = env["hs_pool"].tile([P, KT, S], bf16, tag="hsT", name="hsT_sb")
    for i in range(8):
        nc.sync.dma_start(
            hsT[:, 2 * i:2 * i + 2, :],
            env["hsT_d"][2 * i:2 * i + 2].rearrange("k p s -> p k s"),
        )
    wqs = {}
    for h in range(2):
        wq = env["wq_pool"].tile([P, KT, P], bf16, tag="wq", name="wq_sb")
        nc.sync.dma_start(wq[:], env["wq_d"][h].rearrange("k p m -> p k m"))
        wqs[h] = wq
    return {"hsT": hsT, "wqs": wqs}


def _emit_iteration(env, it, pre, prefetch_next, prev_wo1=None):
    import concourse.tile as tile  # noqa: F401

    nc = env["nc"]
    tc = env["tc"]
    f32, bf16 = env["f32"], env["bf16"]
    mult, add, Exp = env["mult"], env["add"], env["Exp"]
    bass_isa = env["bass_isa"]
    qT, kT, vF = env["qT"], env["kT"], env["vF"]
    attn0 = env["attn0"]
    cs_cc, cs_pm = env["cs_cc"], env["cs_pm"]
    hsT, wqs = pre["hsT"], pre["wqs"]
    wks, wons = env["wks"], env["wons"]
    y_d = env["y_d"]

    ps = env["psum_pool"]

    # ---------------- phase 1: projections + rope ----------------
    with (
        tc.tile_pool(name="rope", bufs=1) as rope_pool,
    ):
        if it == 0:
            # PE warm-up during the initial DMA window; rope tables; caches.
            pw = ps.tile([P, P], f32, tag="av", bufs=2, name="pwarm")
            for i in range(150):
                _mm(nc, pw[:], env["ones128"][:], env["ones128"][:],
                    start=(i == 0), stop=(i == 149), skip_group_check=True)
            wsink = rope_pool.tile([1, 1], f32, tag="wsink", name="wsink")
            nc.vector.tensor_copy(wsink[:], pw[0:1, 0:1])
            nc.sync.dma_start(cs_cc[:], env["cc_d"][:])
            nc.sync.dma_start(cs_pm[:], env["pm_d"][:])
            for g in range(NG):
                nc.sync.dma_start(kT[:, g, 0:C], env["ck_d"][g])
            nc.sync.dma_start(vF[:, 0:C // P, :, :], env["cv_d"][:])

        wv = rope_pool.tile([P, KT, NG * P], bf16, tag="wv", name="wv_sb")
        nc.sync.dma_start(wv[:], env["wv_d"].rearrange("k p n -> p k n"))

        # k projection + rope (new keys go to kT[:, g, C:])
        for g in range(NG):
            pk = ps.tile([P, S], f32, tag="big", bufs=2, name="pk")
            for k in range(KT):
                for n in range(2):
                    _mm(nc, pk[:, n * SH:(n + 1) * SH], wks[g][:, k, :],
                        hsT[:, k, n * SH:(n + 1) * SH],
                        start=(k == 0), stop=(k == KT - 1))
            _rope(nc, rope_pool, f32, pk, kT[:, g, C:C + S], cs_cc, cs_pm,
                  mult)

        # q projection + rope
        for h in range(NH):
            if h in wqs:
                wq = wqs[h]
            else:
                wq = env["wq_pool"].tile([P, KT, P], bf16, tag="wq",
                                         name="wq_sb")
                nc.sync.dma_start(wq[:],
                                  env["wq_d"][h].rearrange("k p m -> p k m"))
            pq = ps.tile([P, S], f32, tag="big", bufs=2, name="pq")
            for k in range(KT):
                for n in range(2):
                    _mm(nc, pq[:, n * SH:(n + 1) * SH], wq[:, k, :],
                        hsT[:, k, n * SH:(n + 1) * SH],
                        start=(k == 0), stop=(k == KT - 1))
            _rope(nc, rope_pool, f32, pq, qT[:, h, :], cs_cc, cs_pm, mult)

        # v projection (natural layout: tokens on partitions)
        for mv in range(S // P):
            pv = ps.tile([P, SH], f32, tag="py", bufs=2, name="pv")
            for k in range(KT):
                _mm(nc, pv[:, 0:NG * P], hsT[:, k, mv * P:(mv + 1) * P],
                    wv[:, k, :], start=(k == 0), stop=(k == KT - 1))
            nc.vector.tensor_copy(vF[:, C // P + mv, :, :], pv[:, 0:NG * P])

    # next iteration's early loads: on the SP queue BEFORE this iteration's
    # y-output triggers, so they stream during this phase 2.
    nxt = _emit_prefetch(env) if prefetch_next else None

    # ---------------- phase 2: attention + output projection ----------------
    with (
        tc.tile_pool(name="probs", bufs=1) as probs_pool,
        tc.tile_pool(name="small", bufs=2) as small_pool,
    ):
        ps2 = ps
        attn1 = env["attn1_pool"].tile([P, NH, SH], bf16, tag="attn1",
                                       name="attn1_sb")

        class WoEmitter:
            """Emits one s-half's output projection as a stream of single
            matmuls so they can be interleaved into the attention pipeline
            as PE filler (the exp chain on ACT otherwise paces PE)."""

            def __init__(self, attn_t, sh):
                self.attn_t = attn_t
                self.sh = sh
                self.jobs = [(n, mt) for n in range(HID // SH)
                             for mt in range(4)]
                self.ji = 0
                self.hi = 0
                self.py = None

            def exhausted(self):
                return self.ji >= len(self.jobs)

            def emit_one(self):
                if self.exhausted():
                    return False
                n, mt = self.jobs[self.ji]
                if self.hi == 0:
                    self.py = ps2.tile([P, SH], f32, tag="py", bufs=2,
                                       name="py")
                h = self.hi
                _mm(nc, self.py[:],
                    self.attn_t[:, h, mt * P:(mt + 1) * P],
                    wons[n][:, h, :], start=(h == 0), stop=(h == NH - 1),
                    skip_group_check=True)
                self.hi += 1
                if self.hi == NH:
                    ysb = small_pool.tile([P, SH], bf16, tag="ysb",
                                          name="ysb")
                    nc.vector.tensor_copy(ysb[:], self.py[:])
                    m = self.sh * 4 + mt
                    nc.sync.dma_start(
                        y_d[m * P:(m + 1) * P, n * SH:(n + 1) * SH], ysb[:],
                    )
                    self.hi = 0
                    self.ji += 1
                return True

            def emit_some(self, k):
                for _ in range(k):
                    if not self.emit_one():
                        return

            def emit_all(self):
                while self.emit_one():
                    pass

        def attention_head(sh, h, attn_t, filler):
            ssl = slice(sh * SH, (sh + 1) * SH)
            g = h // (NH // NG)
            NCG = KC // 2  # chunk groups of 2
            probs = [None] * NCG
            d1 = [None] * NCG
            d2 = [None] * (NCG // 2)
            d3 = [None] * (NCG // 4)
            pav_box = [None]
            den_bc = small_pool.tile([P, SH], f32, tag="denbc", name="den_bc")

            def emit_scores(cg):
                psc = ps2.tile([P, 2, SH], f32, tag="big", bufs=2,
                               name="pscore")
                for j in range(2):
                    c = cg * 2 + j
                    _mm(nc, psc[:, j, :], kT[:, g, c * P:(c + 1) * P],
                        qT[:, h, ssl], start=True, stop=True)
                pt = probs_pool.tile([P, 2, SH], bf16, tag="probs",
                                     bufs=4, name="probs_t")
                nc.scalar.activation(pt[:], psc[:], Exp)
                probs[cg] = pt
                # denominator: DVE pair-sum tree (bf16), one GpSimd
                # partition_all_reduce at the end -> per-s denominator
                # broadcast to every partition; zero PE cycles.
                t1 = probs_pool.tile([P, SH], bf16, tag="d1", bufs=2,
                                     name="d1_t")
                nc.vector.tensor_tensor(t1[:], pt[:, 0, :], pt[:, 1, :], add)
                d1[cg] = t1
                if cg % 2 == 1:
                    t2 = probs_pool.tile([P, SH], bf16, tag="d2", bufs=2,
                                         name="d2_t")
                    nc.vector.tensor_tensor(t2[:], d1[cg - 1][:], t1[:], add)
                    d2[cg // 2] = t2
                if cg % 4 == 3:
                    t3 = probs_pool.tile([P, SH], bf16, tag="d3", bufs=2,
                                         name="d3_t")
                    nc.vector.tensor_tensor(t3[:], d2[cg // 2 - 1][:],
                                            d2[cg // 2][:], add)
                    d3[cg // 4] = t3
                if cg == NCG - 1:
                    t4 = probs_pool.tile([P, SH], bf16, tag="d4", bufs=1,
                                         name="d4_t")
                    nc.vector.tensor_tensor(t4[:], d3[0][:], d3[1][:], add)
                    nc.gpsimd.partition_all_reduce(
                        den_bc[:], t4[:], channels=P,
                        reduce_op=bass_isa.ReduceOp.add)

            def emit_av(cg):
                # lazy: score tiles grab the low PSUM banks, which phase 1's
                # first tiles reuse -- scores free at the last exp, not at
                # the last head's normalize, so the next iteration's
                # projections start ~3.5us earlier.
                if pav_box[0] is None:
                    pav_box[0] = ps2.tile([P, SH], f32, tag="av", bufs=2,
                                          name="pav")
                pav = pav_box[0]
                for j in range(2):
                    c = cg * 2 + j
                    first, last = (c == 0), (c == KC - 1)
                    _mm(nc, pav[:], vF[:, c, g, :], probs[cg][:, j, :],
                        start=first, stop=last, skip_group_check=True)

            # software pipeline: scores 2 groups ahead of AV; up to 2 wo
            # filler matmuls per step keep PE fed while ACT works on exp
            for cg in range(NCG + 2):
                if filler is not None:
                    filler.emit_some(2)
                if cg < NCG:
                    emit_scores(cg)
                if cg >= 2:
                    emit_av(cg - 2)

            # reciprocal of the broadcast denominator, fused into the AV
            # PSUM->SBUF copyback
            rbc = small_pool.tile([P, SH], f32, tag="rbc", name="rbc")
            nc.vector.reciprocal_approx_fast(out=rbc[:], in_=den_bc[:])
            nc.vector.tensor_tensor(attn_t[:, h, :], pav_box[0][:], rbc[:],
                                    mult)

        # s-half 0 heads, filled with the PREVIOUS iteration's s-half-1
        # output projection; s-half 1 heads, filled with this iteration's
        # s-half-0 output projection.
        fill0 = WoEmitter(prev_wo1, 1) if prev_wo1 is not None else None
        for h in range(NH):
            attention_head(0, h, attn0, fill0)
        if fill0 is not None:
            fill0.emit_all()
        fill1 = WoEmitter(attn0, 0)
        for h in range(NH):
            attention_head(1, h, attn1, fill1)
        fill1.emit_all()
        if not prefetch_next:
            # last iteration: its s-half-1 wo has no later home
            WoEmitter(attn1, 1).emit_all()

        # next iteration's kv-cache loads ride the ACT queue: ACT reaches
        # them right after its last exp, and kT/vF are free then too.
        if prefetch_next:
            for g in range(NG):
                nc.scalar.dma_start(kT[:, g, 0:C], env["ck_d"][g])
            nc.scalar.dma_start(vF[:, 0:C // P, :, :], env["cv_d"][:])

    return nxt, attn1


def prep_inputs(hidden_states, freqs_cos, freqs_sin, cache_k, cache_v,
                wq, wk, wv, wo):
    """Shard + pre-transpose the full inputs into 8 per-core input maps."""
    f = np.float32
    b16 = ml_dtypes.bfloat16
    scale = np.float32(1.0 / np.sqrt(D))
    wq_p = (wq.astype(f).reshape(H, D, HID)[:, _PERM, :] * scale)
    wk_p = wk.astype(f).reshape(G, D, HID)[:, _PERM, :]
    wv_r = wv.astype(f).reshape(G, D, HID)

    cc = freqs_cos.astype(f).T          # [64, S]
    ss = freqs_sin.astype(f).T
    cs_cc = np.ascontiguousarray(np.concatenate([cc, cc], axis=0))
    cs_pm = np.ascontiguousarray(np.concatenate([ss, -ss], axis=0))

    in_maps = []
    for b in range(B):
        hsT = np.ascontiguousarray(
            hidden_states[b].astype(f).T.reshape(KT, P, S)).astype(b16)
        for hg in range(2):
            hs_q = slice(hg * NH, (hg + 1) * NH)
            hs_kv = slice(hg * NG, (hg + 1) * NG)
            wqT = wq_p[hs_q].reshape(NH * D, HID).T          # [HID, 1024]
            wqT_t = np.ascontiguousarray(
                wqT.reshape(KT, P, NH, P).transpose(2, 0, 1, 3)).astype(b16)
            wkT = wk_p[hs_kv].reshape(NG * D, HID).T         # [HID, 256]
            wkT_t = np.ascontiguousarray(
                wkT.reshape(KT, P, NG, P).transpose(2, 0, 1, 3)).astype(b16)
            wvT = wv_r[hs_kv].reshape(NG * D, HID).T         # [HID, 256]
            wvT_t = np.ascontiguousarray(
                wvT.reshape(KT, P, NG * P)).astype(b16)
            woT = np.ascontiguousarray(
                wo.astype(f)[:, hg * NH * D:(hg + 1) * NH * D].T
                .reshape(NH, P, HID)).astype(b16)
            ckT = np.ascontiguousarray(
                cache_k[b].astype(f)[:, hs_kv][:, :, _PERM]
                .transpose(1, 2, 0)).astype(b16)
            cvP = np.ascontiguousarray(
                cache_v[b].astype(f)[:, hs_kv]
                .reshape(C // P, P, NG, P).transpose(1, 0, 2, 3)).astype(b16)
            in_maps.append({
                "hsT": hsT, "wqT": wqT_t, "wkT": wkT_t, "wvT": wvT_t,
                "woT": woT, "ckT": ckT, "cvP": cvP,
                "cs_cc": cs_cc, "cs_pm": cs_pm,
            })
    return in_maps


def gather_output(results):
    """Sum the 2 TP partials per batch -> full [B, S, HID] output."""
    out = np.empty((B, S, HID), np.float32)
    for b in range(B):
        out[b] = results[2 * b]["y"] + results[2 * b + 1]["y"]
    return out


def kernel(hidden_states, freqs_cos, freqs_sin, attention_mask,
           cache_k, cache_v, wq, wk, wv, wo):
    # attention_mask is all-zeros by construction (see spec) - unused.
    from concourse.bass_utils import run_bass_kernel_spmd

    nc = build_bass(unroll=1)
    in_maps = prep_inputs(
        np.asarray(hidden_states), np.asarray(freqs_cos), np.asarray(freqs_sin),
        np.asarray(cache_k), np.asarray(cache_v),
        np.asarray(wq), np.asarray(wk), np.asarray(wv), np.asarray(wo))
    res = run_bass_kernel_spmd(nc, in_maps, core_ids=list(range(N_CORES)))
    return gather_output(res.results)


# revision 32
# speedup vs baseline: 1.3748x; 1.0893x over previous
"""BrahmaAttention (GQA prefill with KV cache) on 8 Trainium2 NeuronCores.

Problem: B=4, S=1024, C=1024 (cache), H=16 q-heads, G=4 kv-heads, D=128.
    q = hs @ wq.T ; k = hs @ wk.T ; v = hs @ wv.T
    rope(q, k) (interleaved pairs, positions C..C+S)
    k_full/v_full = concat(cache, new)           # K = 2048 keys
    out = softmax(q k^T / sqrt(D)) @ v_full @ wo.T
(attention_mask is all-zeros by construction - full attention, no masking.)

Sharding: 4-way data parallel over batch x 2-way tensor parallel over heads.
core (b, hg) handles batch b, q-heads hg*8..hg*8+8, kv-heads hg*2..hg*2+2 and
computes a partial output projection over its 1024 hidden columns; the host
sums the two partials per batch (the TP all-reduce done on host at gather).

Host-side prep folded into the shards:
  - 1/sqrt(D) folded into wq.
  - RoPE even/odd interleave permuted to [evens|odds] via wq/wk row
    permutation and cache_k last-dim permutation, so on-chip RoPE is
    half-tile elementwise ops (partitions 0-63 = even, 64-127 = odd lanes).
  - All projection weights pre-transposed/tiled so every DMA is contiguous,
    and shipped in bf16 (PE streams bf16 at the same 1 col/cycle as f32r,
    so bf16 costs nothing on PE and halves DMA + SBUF).

On-chip structure per core (all matmul operands bf16, PSUM f32):
  phase 1: q/k/v projections (PE) + rope (DVE + ACT-copy swap)
  phase 2: scoresT = kT.T @ qT -> PSUM -> exp (ACT) -> probs bf16
           softmax denominator: DVE pair-sum tree over the 16 key chunks,
           then ONE GpSimd partition_all_reduce => per-s denominator already
           broadcast to all partitions (zero PE cycles, zero bcast matmul)
           AV accumulated on PE; normalize fused into the PSUM->SBUF copy
           wo projection; s-half 0's wo tiles interleaved between s-half 1's
           heads to fill PE while ACT paces the exp chain
  cross-iteration software pipeline: hsT/wk/wq tiles double-buffered in
  always-open pools; the NEXT iteration's input DMAs are emitted on the SP
  queue before this iteration's output DMAs, so they stream during phase 2
  instead of serializing at the iteration boundary.  kv-cache loads ride the
  ACT queue (free after its last exp).
"""

import numpy as np
import ml_dtypes

B, S, C, H, G, D = 4, 1024, 1024, 16, 4, 128
HID = H * D
P = 128
NH, NG = 8, 2          # per-core q heads / kv heads
KC = (C + S) // P      # 16 key chunks
KT = 16                # hid contraction tiles
SH = 512               # s-half (PSUM bank free size)
N_CORES = 8

_PERM = np.concatenate([np.arange(0, D, 2), np.arange(1, D, 2)])

_BUILT = {}


def _mm(nc, out, lhsT, rhs, **kw):
    nc.tensor.matmul(out, lhsT, rhs, **kw)


def _rope(nc, pool, f32, psum_in, out_ap, cs_cc, cs_pm, mult):
    """out = psum_in*[cos;cos] + swap_halves(psum_in*[sin;-sin]).

    psum_in is the raw projected [128, S] tile with evens on partitions 0-63
    and odds on 64-127; out gets the roped value in the same layout.
    """
    import concourse.mybir as mybir

    a = pool.tile([P, S], f32, tag="ropeA", name="ropeA")
    b = pool.tile([P, S], f32, tag="ropeB", name="ropeB")
    s = pool.tile([P, S], f32, tag="ropeS", name="ropeS")
    nc.vector.tensor_tensor(a[:], psum_in[:], cs_cc[:], mult)
    nc.vector.tensor_tensor(b[:], psum_in[:], cs_pm[:], mult)
    # swap halves on the scalar engine (idle during phase 1)
    nc.scalar.copy(s[0:64, :], b[64:128, :])
    nc.scalar.copy(s[64:128, :], b[0:64, :])
    nc.vector.tensor_tensor(out_ap, a[:], s[:], mybir.AluOpType.add)


def build_bass(unroll=1):
    """Build + compile the per-core Bass program (identical on all cores)."""
    if unroll in _BUILT:
        return _BUILT[unroll]

    import concourse.mybir as mybir
    import concourse.tile as tile
    import concourse.bass_isa as bass_isa
    from concourse import bacc

    f32 = mybir.dt.float32
    f32r = mybir.dt.float32r
    bf16 = mybir.dt.bfloat16
    mult = mybir.AluOpType.mult
    add = mybir.AluOpType.add
    Exp = mybir.ActivationFunctionType.Exp

    nc = bacc.Bacc("TRN2", target_bir_lowering=False, debug=False)

    hsT_d = nc.dram_tensor("hsT", [KT, P, S], bf16, kind="ExternalInput")
    wq_d = nc.dram_tensor("wqT", [NH, KT, P, P], bf16, kind="ExternalInput")
    wk_d = nc.dram_tensor("wkT", [NG, KT, P, P], bf16, kind="ExternalInput")
    wv_d = nc.dram_tensor("wvT", [KT, P, NG * P], bf16, kind="ExternalInput")
    wo_d = nc.dram_tensor("woT", [NH, P, HID], bf16, kind="ExternalInput")
    ck_d = nc.dram_tensor("ckT", [NG, P, C], bf16, kind="ExternalInput")
    cv_d = nc.dram_tensor("cvP", [P, C // P, NG, P], bf16, kind="ExternalInput")
    cc_d = nc.dram_tensor("cs_cc", [P, S], f32, kind="ExternalInput")
    pm_d = nc.dram_tensor("cs_pm", [P, S], f32, kind="ExternalInput")
    y_d = nc.dram_tensor("y", [S, HID], bf16, kind="ExternalOutput")

    with tile.TileContext(nc) as tc:
        with (
            tc.tile_pool(name="const", bufs=1) as const,
            tc.tile_pool(name="hs", bufs=2) as hs_pool,
            tc.tile_pool(name="wq_pool", bufs=2) as wq_pool,
            tc.tile_pool(name="wk_pool", bufs=2) as wk_pool,
            tc.tile_pool(name="a1", bufs=2) as a1_pool,
            tc.tile_pool(name="persist", bufs=1) as persist,
            tc.tile_pool(name="psum", bufs=1, space="PSUM") as psum_pool,
        ):
            ones_f = const.tile([P, P], f32, name="ones_f")
            nc.any.memset(ones_f[:], 1.0)
            ones128 = const.tile([P, P], f32r, name="ones128")
            nc.vector.tensor_copy(ones128[:], ones_f[:])
            cs_cc = const.tile([P, S], f32, name="cs_cc")
            cs_pm = const.tile([P, S], f32, name="cs_pm")

            qT = persist.tile([P, NH, S], bf16, name="qT")
            kT = persist.tile([P, NG, C + S], bf16, name="kT")
            vF = persist.tile([P, KC, NG, P], bf16, name="vF")
            attn0 = persist.tile([P, NH, SH], bf16, name="attn0")

            # weights identical across unrolled iterations: wk and wo are
            # loaded once up front and never reloaded.
            wks = []
            for g in range(NG):
                wk = wk_pool.tile([P, KT, P], bf16, tag="wk", name="wk_sb")
                nc.sync.dma_start(wk[:],
                                  wk_d[g].rearrange("k p m -> p k m"))
                wks.append(wk)
            wons = [persist.tile([P, NH, SH], bf16, tag=f"won{n}",
                                 name="won") for n in range(HID // SH)]

            env = dict(
                nc=nc, tc=tc, f32=f32, f32r=f32r, bf16=bf16, mult=mult,
                add=add, Exp=Exp, bass_isa=bass_isa,
                hsT_d=hsT_d, wq_d=wq_d, wk_d=wk_d, wv_d=wv_d, wo_d=wo_d,
                ck_d=ck_d, cv_d=cv_d, cc_d=cc_d, pm_d=pm_d, y_d=y_d,
                hs_pool=hs_pool, wq_pool=wq_pool, wk_pool=wk_pool,
                qT=qT, kT=kT, vF=vF, attn0=attn0,
                wks=wks, wons=wons,
                ones128=ones128, cs_cc=cs_cc, cs_pm=cs_pm,
                attn1_pool=a1_pool, psum_pool=psum_pool,
            )

            # iteration 0's phase-1 loads, emitted cold; wo weights after
            # them in the queue (first needed only in phase 2)
            pre = _emit_prefetch(env)
            for n in range(HID // SH):
                nc.sync.dma_start(
                    wons[n][:],
                    wo_d[:, :, n * SH:(n + 1) * SH]
                    .rearrange("h p n -> p h n"),
                )
            prev_wo1 = None
            for it in range(unroll):
                pre, prev_wo1 = _emit_iteration(
                    env, it, pre, prefetch_next=(it + 1 < unroll),
                    prev_wo1=prev_wo1)

    nc.compile()
    _BUILT[unroll] = nc
    return nc


def _emit_prefetch(env):
    """Emit the early input loads for one iteration on the SP queue:
    hsT (all 16 k-tiles) and wq heads 0-1.  Returns the tiles for the
    consuming iteration."""
    nc = env["nc"]
    bf16 = env["bf16"]
    hsT = env["hs_pool"].tile([P, KT, S], bf16, tag="hsT", name="hsT_sb")
    for i in range(8):
        nc.sync.dma_start(
            hsT[:, 2 * i:2 * i + 2, :],
            env["hsT_d"][2 * i:2 * i + 2].rearrange("k p s -> p k s"),
        )
    wqs = {}
    for h in range(2):
        wq = env["wq_pool"].tile([P, KT, P], bf16, tag="wq", name="wq_sb")
        nc.sync.dma_start(wq[:], env["wq_d"][h].rearrange("k p m -> p k m"))
        wqs[h] = wq
    return {"hsT": hsT, "wqs": wqs}


def _emit_iteration(env, it, pre, prefetch_next, prev_wo1=None):
    import concourse.tile as tile  # noqa: F401

    nc = env["nc"]
    tc = env["tc"]
    f32, bf16 = env["f32"], env["bf16"]
    mult, add, Exp = env["mult"], env["add"], env["Exp"]
    bass_isa = env["bass_isa"]
    qT, kT, vF = env["qT"], env["kT"], env["vF"]
    attn0 = env["attn0"]
    cs_cc, cs_pm = env["cs_cc"], env["cs_pm"]
    hsT, wqs = pre["hsT"], pre["wqs"]
    wks, wons = env["wks"], env["wons"]
    y_d = env["y_d"]

    ps = env["psum_pool"]

    # ---------------- phase 1: projections + rope ----------------
    with (
        tc.tile_pool(name="rope", bufs=1) as rope_pool,
    ):
        if it == 0:
            # PE warm-up during the initial DMA window; rope tables; caches.
            pw = ps.tile([P, P], f32, tag="av", bufs=2, name="pwarm")
            for i in range(150):
                _mm(nc, pw[:], env["ones128"][:], env["ones128"][:],
                    start=(i == 0), stop=(i == 149), skip_group_check=True)
            wsink = rope_pool.tile([1, 1], f32, tag="wsink", name="wsink")
            nc.vector.tensor_copy(wsink[:], pw[0:1, 0:1])
            nc.sync.dma_start(cs_cc[:], env["cc_d"][:])
            nc.sync.dma_start(cs_pm[:], env["pm_d"][:])
            for g in range(NG):
                nc.sync.dma_start(kT[:, g, 0:C], env["ck_d"][g])
            nc.sync.dma_start(vF[:, 0:C // P, :, :], env["cv_d"][:])

        wv = rope_pool.tile([P, KT, NG * P], bf16, tag="wv", name="wv_sb")
        nc.sync.dma_start(wv[:], env["wv_d"].rearrange("k p n -> p k n"))

        # k projection + rope (new keys go to kT[:, g, C:])
        for g in range(NG):
            pk = ps.tile([P, S], f32, tag="big", bufs=2, name="pk")
            for k in range(KT):
                for n in range(2):
                    _mm(nc, pk[:, n * SH:(n + 1) * SH], wks[g][:, k, :],
                        hsT[:, k, n * SH:(n + 1) * SH],
                        start=(k == 0), stop=(k == KT - 1))
            _rope(nc, rope_pool, f32, pk, kT[:, g, C:C + S], cs_cc, cs_pm,
                  mult)

        # q projection + rope
        for h in range(NH):
            if h in wqs:
                wq = wqs[h]
            else:
                wq = env["wq_pool"].tile([P, KT, P], bf16, tag="wq",
                                         name="wq_sb")
                nc.sync.dma_start(wq[:],
                                  env["wq_d"][h].rearrange("k p m -> p k m"))
            pq = ps.tile([P, S], f32, tag="big", bufs=2, name="pq")
            for k in range(KT):
                for n in range(2):
                    _mm(nc, pq[:, n * SH:(n + 1) * SH], wq[:, k, :],
                        hsT[:, k, n * SH:(n + 1) * SH],
                        start=(k == 0), stop=(k == KT - 1))
            _rope(nc, rope_pool, f32, pq, qT[:, h, :], cs_cc, cs_pm, mult)

        # v projection (natural layout: tokens on partitions)
        for mv in range(S // P):
            pv = ps.tile([P, SH], f32, tag="py", bufs=2, name="pv")
            for k in range(KT):
                _mm(nc, pv[:, 0:NG * P], hsT[:, k, mv * P:(mv + 1) * P],
                    wv[:, k, :], start=(k == 0), stop=(k == KT - 1))
            nc.vector.tensor_copy(vF[:, C // P + mv, :, :], pv[:, 0:NG * P])

    # next iteration's early loads: on the SP queue BEFORE this iteration's
    # y-output triggers, so they stream during this phase 2.
    nxt = _emit_prefetch(env) if prefetch_next else None

    # ---------------- phase 2: attention + output projection ----------------
    with (
        tc.tile_pool(name="probs", bufs=1) as probs_pool,
        tc.tile_pool(name="small", bufs=2) as small_pool,
    ):
        ps2 = ps
        attn1 = env["attn1_pool"].tile([P, NH, SH], bf16, tag="attn1",
                                       name="attn1_sb")

        class WoEmitter:
            """Emits one s-half's output projection as a stream of single
            matmuls so they can be interleaved into the attention pipeline
            as PE filler (the exp chain on ACT otherwise paces PE)."""

            def __init__(self, attn_t, sh):
                self.attn_t = attn_t
                self.sh = sh
                self.jobs = [(n, mt) for n in range(HID // SH)
                             for mt in range(4)]
                self.ji = 0
                self.hi = 0
                self.py = None

            def exhausted(self):
                return self.ji >= len(self.jobs)

            def emit_one(self):
                if self.exhausted():
                    return False
                n, mt = self.jobs[self.ji]
                if self.hi == 0:
                    self.py = ps2.tile([P, SH], f32, tag="py", bufs=2,
                                       name="py")
                h = self.hi
                _mm(nc, self.py[:],
                    self.attn_t[:, h, mt * P:(mt + 1) * P],
                    wons[n][:, h, :], start=(h == 0), stop=(h == NH - 1),
                    skip_group_check=True)
                self.hi += 1
                if self.hi == NH:
                    ysb = small_pool.tile([P, SH], bf16, tag="ysb",
                                          name="ysb")
                    nc.vector.tensor_copy(ysb[:], self.py[:])
                    m = self.sh * 4 + mt
                    nc.sync.dma_start(
                        y_d[m * P:(m + 1) * P, n * SH:(n + 1) * SH], ysb[:],
                    )
                    self.hi = 0
                    self.ji += 1
                return True

            def emit_some(self, k):
                for _ in range(k):
                    if not self.emit_one():
                        return

            def emit_all(self):
                while self.emit_one():
                    pass

        def attention_head(sh, h, attn_t, filler):
            ssl = slice(sh * SH, (sh + 1) * SH)
            g = h // (NH // NG)
            NCG = KC // 2  # chunk groups of 2
            probs = [None] * NCG
            d1 = [None] * NCG
            d2 = [None] * (NCG // 2)
            d3 = [None] * (NCG // 4)
            pav_box = [None]
            den_bc = small_pool.tile([P, SH], f32, tag="denbc", name="den_bc")

            def emit_scores(cg):
                psc = ps2.tile([P, 2, SH], f32, tag="big", bufs=2,
                               name="pscore")
                for j in range(2):
                    c = cg * 2 + j
                    _mm(nc, psc[:, j, :], kT[:, g, c * P:(c + 1) * P],
                        qT[:, h, ssl], start=True, stop=True)
                pt = probs_pool.tile([P, 2, SH], bf16, tag="probs",
                                     bufs=4, name="probs_t")
                nc.scalar.activation(pt[:], psc[:], Exp)
                probs[cg] = pt
                # denominator: DVE pair-sum tree (bf16), one GpSimd
                # partition_all_reduce at the end -> per-s denominator
                # broadcast to every partition; zero PE cycles.
                t1 = probs_pool.tile([P, SH], bf16, tag="d1", bufs=2,
                                     name="d1_t")
                nc.vector.tensor_tensor(t1[:], pt[:, 0, :], pt[:, 1, :], add)
                d1[cg] = t1
                if cg % 2 == 1:
                    t2 = probs_pool.tile([P, SH], bf16, tag="d2", bufs=2,
                                         name="d2_t")
                    nc.vector.tensor_tensor(t2[:], d1[cg - 1][:], t1[:], add)
                    d2[cg // 2] = t2
                if cg % 4 == 3:
                    t3 = probs_pool.tile([P, SH], bf16, tag="d3", bufs=2,
                                         name="d3_t")
                    nc.vector.tensor_tensor(t3[:], d2[cg // 2 - 1][:],
                                            d2[cg // 2][:], add)
                    d3[cg // 4] = t3
                if cg == NCG - 1:
                    t4 = probs_pool.tile([P, SH], bf16, tag="d4", bufs=1,
                                         name="d4_t")
                    nc.vector.tensor_tensor(t4[:], d3[0][:], d3[1][:], add)
                    nc.gpsimd.partition_all_reduce(
                        den_bc[:], t4[:], channels=P,
                        reduce_op=bass_isa.ReduceOp.add)

            def emit_av(cg):
                # lazy: score tiles grab the low PSUM banks, which phase 1's
                # first tiles reuse -- scores free at the last exp, not at
                # the last head's normalize, so the next iteration's
                # projections start ~3.5us earlier.
                if pav_box[0] is None:
                    pav_box[0] = ps2.tile([P, SH], f32, tag="av", bufs=2,
                                          name="pav")
                pav = pav_box[0]
                for j in range(2):
                    c = cg * 2 + j
                    first, last = (c == 0), (c == KC - 1)
                    _mm(nc, pav[:], vF[:, c, g, :], probs[cg][:, j, :],
                        start=first, stop=last, skip_group_check=True)

            # software pipeline: scores 2 groups ahead of AV; up to 2 wo
            # filler matmuls per step keep PE fed while ACT works on exp
            for cg in range(NCG + 2):
                if filler is not None:
                    filler.emit_some(2)
                if cg < NCG:
                    emit_scores(cg)
                if cg >= 2:
                    emit_av(cg - 2)

            # reciprocal of the broadcast denominator, fused into the AV
            # PSUM->SBUF copyback
            rbc = small_pool.tile([P, SH], f32, tag="rbc", name="rbc")
            nc.vector.reciprocal_approx_fast(out=rbc[:], in_=den_bc[:])
            nc.vector.tensor_tensor(attn_t[:, h, :], pav_box[0][:], rbc[:],
                                    mult)

        # s-half 0 heads, filled with the PREVIOUS iteration's s-half-1
        # output projection; s-half 1 heads, filled with this iteration's
        # s-half-0 output projection.
        fill0 = WoEmitter(prev_wo1, 1) if prev_wo1 is not None else None
        for h in range(NH):
            attention_head(0, h, attn0, fill0)
        if fill0 is not None:
            fill0.emit_all()
        fill1 = WoEmitter(attn0, 0)
        for h in range(NH):
            attention_head(1, h, attn1, fill1)
        fill1.emit_all()
        if not prefetch_next:
            # last iteration: its s-half-1 wo has no later home
            WoEmitter(attn1, 1).emit_all()

        # next iteration's kv-cache loads ride the ACT queue: ACT reaches
        # them right after its last exp, and kT/vF are free then too.
        if prefetch_next:
            for g in range(NG):
                nc.scalar.dma_start(kT[:, g, 0:C], env["ck_d"][g])
            nc.scalar.dma_start(vF[:, 0:C // P, :, :], env["cv_d"][:])

    return nxt, attn1


def prep_inputs(hidden_states, freqs_cos, freqs_sin, cache_k, cache_v,
                wq, wk, wv, wo):
    """Shard + pre-transpose the full inputs into 8 per-core input maps."""
    f = np.float32
    b16 = ml_dtypes.bfloat16
    scale = np.float32(1.0 / np.sqrt(D))
    wq_p = (wq.astype(f).reshape(H, D, HID)[:, _PERM, :] * scale)
    wk_p = wk.astype(f).reshape(G, D, HID)[:, _PERM, :]
    wv_r = wv.astype(f).reshape(G, D, HID)

    cc = freqs_cos.astype(f).T          # [64, S]
    ss = freqs_sin.astype(f).T
    cs_cc = np.ascontiguousarray(np.concatenate([cc, cc], axis=0))
    cs_pm = np.ascontiguousarray(np.concatenate([ss, -ss], axis=0))

    in_maps = []
    for b in range(B):
        hsT = np.ascontiguousarray(
            hidden_states[b].astype(f).T.reshape(KT, P, S)).astype(b16)
        for hg in range(2):
            hs_q = slice(hg * NH, (hg + 1) * NH)
            hs_kv = slice(hg * NG, (hg + 1) * NG)
            wqT = wq_p[hs_q].reshape(NH * D, HID).T          # [HID, 1024]
            wqT_t = np.ascontiguousarray(
                wqT.reshape(KT, P, NH, P).transpose(2, 0, 1, 3)).astype(b16)
            wkT = wk_p[hs_kv].reshape(NG * D, HID).T         # [HID, 256]
            wkT_t = np.ascontiguousarray(
                wkT.reshape(KT, P, NG, P).transpose(2, 0, 1, 3)).astype(b16)
            wvT = wv_r[hs_kv].reshape(NG * D, HID).T         # [HID, 256]
            wvT_t = np.ascontiguousarray(
                wvT.reshape(KT, P, NG * P)).astype(b16)
            woT = np.ascontiguousarray(
                wo.astype(f)[:, hg * NH * D:(hg + 1) * NH * D].T
                .reshape(NH, P, HID)).astype(b16)
            ckT = np.ascontiguousarray(
                cache_k[b].astype(f)[:, hs_kv][:, :, _PERM]
                .transpose(1, 2, 0)).astype(b16)
            cvP = np.ascontiguousarray(
                cache_v[b].astype(f)[:, hs_kv]
                .reshape(C // P, P, NG, P).transpose(1, 0, 2, 3)).astype(b16)
            in_maps.append({
                "hsT": hsT, "wqT": wqT_t, "wkT": wkT_t, "wvT": wvT_t,
                "woT": woT, "ckT": ckT, "cvP": cvP,
                "cs_cc": cs_cc, "cs_pm": cs_pm,
            })
    return in_maps


def gather_output(results):
    """Sum the 2 TP partials per batch -> full [B, S, HID] output."""
    out = np.empty((B, S, HID), np.float32)
    for b in range(B):
        out[b] = results[2 * b]["y"] + results[2 * b + 1]["y"]
    return out


def kernel(hidden_states, freqs_cos, freqs_sin, attention_mask,
           cache_k, cache_v, wq, wk, wv, wo):
    # attention_mask is all-zeros by construction (see spec) - unused.
    from concourse.bass_utils import run_bass_kernel_spmd

    nc = build_bass(unroll=1)
    in_maps = prep_inputs(
        np.asarray(hidden_states), np.asarray(freqs_cos), np.asarray(freqs_sin),
        np.asarray(cache_k), np.asarray(cache_v),
        np.asarray(wq), np.asarray(wk), np.asarray(wv), np.asarray(wo))
    res = run_bass_kernel_spmd(nc, in_maps, core_ids=list(range(N_CORES)))
    return gather_output(res.results)
